# revision 1
# baseline (speedup 1.0000x reference)
"""DiffAttention TRN2 kernel: 8-way (batch x seq-half) sharded, zero collectives.

Shapes: x [4, 4096, 1024], H=16 heads, head organized as (h, 2 branches, 32 dims),
v head dim 64. Each core handles one (batch, query-half): 2048 query rows,
recomputes K/V for its batch's full 4096 keys (cheap vs any collective).

Layout strategy (everything transposed so contractions sit on partitions):
  - qkv phase: Q^T [1024, 2048], K^T [1024, 4096] (c on partitions) and
    V [4096, 1024] (tokens on partitions) written to DRAM scratch.
  - attention per (head, branch): S^T tiles [128 k, 1024 q] in PSUM from
    lhsT=K_h^T slice [32, 128], rhs=Q_h^T [32, 512] (f32r full-rate matmuls);
    exp on ACT (scale=1/sqrt(hd) folded in); PV accumulates
    O^T [65, 2048] with lhsT=V_aug [128, 65] (65th col = ones -> softmax
    denominators land in row 64 free of charge).
  - DiffAttn combine + RMS norm done column-wise on O^T with row-broadcasts
    done via SBUF->SBUF DMA; RMS col-sums via ones-vector matmul.
  - proj: lhsT = o^T accumulator tile directly, bias via K=1 ones matmul.
"""

import os
import sys

import numpy as np

for p in ("/opt/trn_rl_repo",):
    if p not in sys.path:
        sys.path.insert(0, p)

import concourse.bass as bass
import concourse.bacc as bacc_mod
import concourse.mybir as mybir
from concourse.bass_utils import run_bass_kernel_spmd
from concourse.tile import TileContext

F32 = mybir.dt.float32
F32R = mybir.dt.float32r

B, N, DIM, H, HD = 4, 4096, 1024, 16, 32
VD = 2 * HD  # 64, per-head v dim
NQ = 2048  # query rows per core
NCORES = 8
LAMBDA_INIT = 0.2
EPS = 1e-5
SCALE = HD ** -0.5

_CACHE = {}


def _r(ap):
    return ap.bitcast(F32R)


def build_nc(lam: float):
    nc = bacc_mod.Bacc(None, target_bir_lowering=False)

    xbt = nc.declare_dram_parameter("xbt", [DIM, N], F32, isOutput=False)
    wqkvt = nc.declare_dram_parameter("wqkvt", [DIM, 3 * DIM], F32, isOutput=False)
    wprojt = nc.declare_dram_parameter("wprojt", [DIM, DIM], F32, isOutput=False)
    bproj = nc.declare_dram_parameter("bproj", [1, DIM], F32, isOutput=False)
    weff = nc.declare_dram_parameter("weff", [VD, 1], F32, isOutput=False)
    y = nc.declare_dram_parameter("y", [NQ, DIM], F32, isOutput=True)

    qt_s = nc.dram_tensor("qt_scratch", [DIM, NQ], F32)
    kt_s = nc.dram_tensor("kt_scratch", [DIM, N], F32)
    v_s = nc.dram_tensor("v_scratch", [N, DIM], F32)

    KT = N // 128  # 32 key tiles
    CIN = DIM // 128  # 8 contraction tiles

    with nc.allow_low_precision(reason="f32r tiles are bit-identical fp32"), \
         TileContext(nc) as tc:
        # ---------------- persistent pools ----------------
        with (
            tc.tile_pool(name="const", bufs=1) as constp,
            tc.tile_pool(name="psA", bufs=2, space="PSUM") as psA,  # [128,1024] slots
            tc.tile_pool(name="psO", bufs=1, space="PSUM") as psO,  # [65,2048] slot
        ):
            ones64 = constp.tile([VD, 1], F32R)
            nc.vector.memset(ones64.bitcast(F32), 1.0)
            ones1 = constp.tile([1, 128], F32R)
            nc.vector.memset(ones1.bitcast(F32), 1.0)
            ones_vd = constp.tile([1, VD], F32R)
            nc.vector.memset(ones_vd.bitcast(F32), 1.0)
            eps_t = constp.tile([1, 1], F32)
            nc.vector.memset(eps_t, EPS)
            weff_t = constp.tile([VD, 1], F32)
            nc.sync.dma_start(out=weff_t, in_=weff[:, :])
            # ================= phase A: qkv =================
            with (
                tc.tile_pool(name="xbt_p", bufs=1) as xbtp,
                tc.tile_pool(name="wq_p", bufs=4) as wqp,
                tc.tile_pool(name="drain_p", bufs=3) as drp,
            ):
                xb = xbtp.tile([128, CIN, N], F32R)
                nc.sync.dma_start(
                    out=xb, in_=xbt[:, :].rearrange("(t p) n -> p t n", p=128).bitcast(F32R)
                )
                # --- Q^T and K^T co-tiles ---
                for co in range(2 * CIN):  # 0..7 Q, 8..15 K
                    is_q = co < CIN
                    tok = NQ if is_q else N
                    for ch in range(tok // 1024):
                        ps = psA.tile([128, 1024], F32, tag="ps")
                        for ci in range(CIN):
                            wt = wqp.tile([128, 128], F32R, tag="w")
                            nc.sync.dma_start(
                                out=wt,
                                in_=wqkvt[ci * 128:(ci + 1) * 128, co * 128:(co + 1) * 128].bitcast(F32R),
                            )
                            for sb in range(2):
                                nc.tensor.matmul(
                                    ps[:, sb * 512:(sb + 1) * 512],
                                    _r(wt),
                                    _r(xb[:, ci, ch * 1024 + sb * 512: ch * 1024 + (sb + 1) * 512]),
                                    start=(ci == 0),
                                    stop=(ci == CIN - 1),
                                )
                        dr = drp.tile([128, 1024], F32, tag="dr")
                        nc.vector.tensor_copy(dr, ps)
                        dst = qt_s if is_q else kt_s
                        coo = co if is_q else co - CIN
                        nc.sync.dma_start(
                            out=dst[coo * 128:(coo + 1) * 128, ch * 1024:(ch + 1) * 1024],
                            in_=dr,
                        )
                # --- V (untransposed) in c-chunks of 256 ---
                with tc.tile_pool(name="wv_p", bufs=8) as wvp:
                    for cc in range(DIM // 256):
                        wv_tiles = []
                        for ci in range(CIN):
                            wv = wvp.tile([128, 256], F32R, tag="wv")
                            nc.sync.dma_start(
                                out=wv,
                                in_=wqkvt[ci * 128:(ci + 1) * 128,
                                          2 * DIM + cc * 256: 2 * DIM + (cc + 1) * 256].bitcast(F32R),
                            )
                            wv_tiles.append(wv)
                        for kt in range(KT):
                            psv = psA.tile([128, 256], F32, tag="ps")
                            for ci in range(CIN):
                                nc.tensor.matmul(
                                    psv,
                                    _r(xb[:, ci, kt * 128:(kt + 1) * 128]),
                                    _r(wv_tiles[ci]),
                                    start=(ci == 0),
                                    stop=(ci == CIN - 1),
                                )
                            drv = drp.tile([128, 256], F32, tag="dr")
                            nc.vector.tensor_copy(drv, psv)
                            nc.sync.dma_start(
                                out=v_s[kt * 128:(kt + 1) * 128, cc * 256:(cc + 1) * 256],
                                in_=drv,
                            )

            # ================= phase B: attention =================
            with tc.tile_pool(name="ot", bufs=1) as otp:
              # o^T accumulator: [128 part, 8 cin-tiles, 2048 q] = 64KB/part
              ot_acc = otp.tile([128, CIN, NQ], F32R)
              with (
                tc.tile_pool(name="qk_p", bufs=2) as qkp,
                tc.tile_pool(name="vh_p", bufs=2) as vhp,
                tc.tile_pool(name="es_p", bufs=2) as esp,
                tc.tile_pool(name="o1_p", bufs=2) as o1p,
                tc.tile_pool(name="row_p", bufs=3) as rowp,
            ):
                  for h in range(H):
                      # V_aug for this head: [128, KT, 65]; col 64 = ones
                      vh = vhp.tile([128, KT, 65], F32R, tag="vh")
                      for kt0 in range(KT):
                          nc.sync.dma_start(
                              out=vh[:, kt0, 0:VD],
                              in_=v_s[kt0 * 128:(kt0 + 1) * 128,
                                      h * VD:(h + 1) * VD].bitcast(F32R),
                          )
                      nc.vector.memset(vh[:, :, VD:65].bitcast(F32), 1.0)

                      o1s = None
                      for br in range(2):
                          r0 = h * VD + br * HD
                          # Q_h^T packed [128, NQ//512 groups? -> [32*4, ...]:
                          # pack 4 column-tiles onto partitions: [128, NQ//512, 128]
                          qh = qkp.tile([128, NQ], F32R, tag="qh")
                          kh = qkp.tile([128, N // 256, 128], F32R, tag="kh")
                          for rrp in range(2):
                              nc.sync.dma_start(
                                  out=qh[rrp * 64:rrp * 64 + HD, :],
                                  in_=qt_s[r0:r0 + HD, :].bitcast(F32R),
                              )
                          for bq in range(2):
                              nc.sync.dma_start(
                                  out=kh[bq * 64:bq * 64 + HD, :, :],
                                  in_=kt_s[r0:r0 + HD, :].rearrange(
                                      "d (g b t) -> d g b t", b=2, t=128
                                  )[:, :, bq, :].bitcast(F32R),
                              )
                          o_ps = psO.tile([65, NQ], F32, tag="o")
                          for qc in range(NQ // 1024):
                              for kt in range(KT):
                                  sps = psA.tile([128, 1024], F32, tag="ps")
                                  kb = 64 * (kt % 2)
                                  klhs = kh[kb:kb + HD, kt // 2, :]
                                  for sb in range(2):
                                      qi = 2 * qc + sb
                                      nc.tensor.matmul(
                                          sps[:, sb * 512:(sb + 1) * 512],
                                          _r(klhs),
                                          _r(qh[kb:kb + HD, qi * 512:(qi + 1) * 512]),
                                          start=True,
                                          stop=True,
                                      )
                                  es = esp.tile([128, 1024], F32R, tag="es")
                                  nc.scalar.activation(
                                      es, sps, mybir.ActivationFunctionType.Exp,
                                      scale=SCALE,
                                  )
                                  for sb in range(2):
                                      nc.tensor.matmul(
                                          o_ps[:, qc * 1024 + sb * 512: qc * 1024 + (sb + 1) * 512],
                                          _r(vh[:, kt, :]),
                                          _r(es[:, sb * 512:(sb + 1) * 512]),
                                          start=(kt == 0),
                                          stop=(kt == KT - 1),
                                      )
                          if br == 0:
                              o1s = o1p.tile([65, NQ], F32, tag="o1", bufs=3)
                              nc.vector.tensor_copy(o1s, o_ps)
                          else:
                            o2s = o1p.tile([65, NQ], F32, tag="o1", bufs=3)
                            nc.vector.tensor_copy(o2s, o_ps)
                            # t = lam * s1 / s2   [1, NQ]
                            s2r = rowp.tile([1, NQ], F32, tag="row")
                            nc.vector.reciprocal(s2r, o2s[64:65, :])
                            trow = rowp.tile([1, NQ], F32R, tag="row")
                            nc.vector.tensor_scalar_mul(trow, o1s[64:65, :], lam)
                            nc.vector.tensor_mul(trow, trow, s2r)
                            od = o1p.tile([VD, NQ], F32, tag="tmp")
                            sq = o1p.tile([VD, NQ], F32R, tag="tbc")
                            msr = rowp.tile([1, NQ], F32, tag="row")
                            for i in range(NQ // 512):
                                sl = slice(i * 512, (i + 1) * 512)
                                tps = psA.tile([VD, 512], F32, tag="ps")
                                nc.tensor.matmul(
                                    tps, _r(ones_vd), _r(trow[:, sl]),
                                    start=True, stop=True,
                                )
                                nc.vector.tensor_mul(od[:, sl], tps, o2s[0:VD, sl])
                                nc.vector.tensor_sub(od[:, sl], o1s[0:VD, sl], od[:, sl])
                                # ---- RMS norm (scale-invariance: od = s1 * o) ----
                                nc.vector.tensor_mul(sq[:, sl], od[:, sl], od[:, sl])
                                rps = psA.tile([1, 512], F32, tag="ps")
                                nc.tensor.matmul(
                                    rps, _r(ones64), _r(sq[:, sl]),
                                    start=True, stop=True,
                                )
                                nc.vector.tensor_copy(msr[:, sl], rps)
                            # ms_true = msr / (VD * s1^2); sd = sqrt(ms_true+eps)
                            s1sq = rowp.tile([1, NQ], F32, tag="row")
                            nc.vector.tensor_mul(s1sq, o1s[64:65, :], o1s[64:65, :])
                            nc.vector.reciprocal(s1sq, s1sq)
                            nc.vector.tensor_mul(msr, msr, s1sq)
                            sd = rowp.tile([1, NQ], F32, tag="row")
                            nc.scalar.activation(
                                sd, msr, mybir.ActivationFunctionType.Sqrt,
                                bias=eps_t, scale=1.0 / VD,
                            )
                            rr = rowp.tile([1, NQ], F32R, tag="row")
                            nc.vector.reciprocal(rr, sd)
                            # od = s1*o -> o_normed = od * rr / s1: fold 1/s1 in
                            s1r = rowp.tile([1, NQ], F32, tag="row")
                            nc.vector.reciprocal(s1r, o1s[64:65, :])
                            nc.vector.tensor_mul(rr, rr, s1r)
                            p0 = (h % 2) * VD
                            for i in range(NQ // 512):
                                sl = slice(i * 512, (i + 1) * 512)
                                rbs = psA.tile([VD, 512], F32, tag="ps")
                                nc.tensor.matmul(
                                    rbs, _r(ones_vd), _r(rr[:, sl]),
                                    start=True, stop=True,
                                )
                                nc.vector.tensor_mul(od[:, sl], od[:, sl], rbs)
                                nc.vector.tensor_scalar_mul(
                                    ot_acc[p0:p0 + VD, h // 2, sl], od[:, sl], weff_t,
                                )

              # ================= phase C: proj =================
              with (
                  tc.tile_pool(name="wp_p", bufs=1) as wpp,
                  tc.tile_pool(name="yd_p", bufs=3) as ydp,
              ):
                  wp = wpp.tile([128, CIN, DIM], F32R)
                  nc.sync.dma_start(
                      out=wp, in_=wprojt[:, :].rearrange("(t p) n -> p t n", p=128).bitcast(F32R)
                  )
                  bp = wpp.tile([1, DIM], F32R)
                  nc.sync.dma_start(out=bp, in_=bproj[:, :].bitcast(F32R))
                  for qt in range(NQ // 128):
                      yps = psA.tile([128, 1024], F32, tag="ps")
                      for sb in range(2):
                          for ci in range(CIN):
                              nc.tensor.matmul(
                                  yps[:, sb * 512:(sb + 1) * 512],
                                  _r(ot_acc[:, ci, qt * 128:(qt + 1) * 128]),
                                  _r(wp[:, ci, sb * 512:(sb + 1) * 512]),
                                  start=(ci == 0),
                                  stop=False,
                              )
                          nc.tensor.matmul(
                              yps[:, sb * 512:(sb + 1) * 512],
                              _r(ones1),
                              _r(bp[:, sb * 512:(sb + 1) * 512]),
                              start=False,
                              stop=True,
                          )
                      yd = ydp.tile([128, 1024], F32, tag="yd")
                      nc.vector.tensor_copy(yd, yps)
                      nc.sync.dma_start(
                          out=y[qt * 128:(qt + 1) * 128, :], in_=yd
                      )
    nc.finalize()
    return nc


def kernel(x, w_qkv, w_proj, b_proj, lambda_q1, lambda_k1, lambda_q2,
           lambda_k2, sub_norm_w):
    x = np.asarray(x, np.float32)
    lam = float(
        np.exp(np.sum(np.float64(lambda_q1) * np.float64(lambda_k1)))
        - np.exp(np.sum(np.float64(lambda_q2) * np.float64(lambda_k2)))
        + LAMBDA_INIT
    )
    wqkvt = np.ascontiguousarray(np.asarray(w_qkv, np.float32).T)
    wprojt = np.ascontiguousarray(np.asarray(w_proj, np.float32).T)
    bp = np.asarray(b_proj, np.float32).reshape(1, DIM)
    weff = (np.asarray(sub_norm_w, np.float32) * (1.0 - LAMBDA_INIT)).reshape(VD, 1)

    key = round(lam, 12)
    if key not in _CACHE:
        _CACHE[key] = build_nc(lam)
    nc = _CACHE[key]

    in_maps = []
    for c in range(NCORES):
        b, half = c // 2, c % 2
        xt = np.asarray(x[b].T)  # [DIM, N]
        if half == 1:  # query rows first
            xt = np.concatenate([xt[:, NQ:], xt[:, :NQ]], axis=1)
        in_maps.append({
            "xbt": np.ascontiguousarray(xt),
            "wqkvt": wqkvt,
            "wprojt": wprojt,
            "bproj": bp,
            "weff": weff,
        })
    res = run_bass_kernel_spmd(nc, in_maps, list(range(NCORES)))
    out = np.empty((B, N, DIM), np.float32)
    for c in range(NCORES):
        b, half = c // 2, c % 2
        out[b, half * NQ:(half + 1) * NQ, :] = res.results[c]["y"]
    return out



# revision 10
# speedup vs baseline: 1.9261x; 1.9261x over previous
"""DiffAttention TRN2 kernel: 8-way (batch x seq-half) sharded, zero collectives.

v2 pipeline (ACT-exp is the roofline: 268M exps/core ~= 2.05ms):
  - Phase A: qkv projections to DRAM scratch (Q^T, K^T, V), token-quartered.
  - Phase B attention, combo-major ((head,branch) = combo, in 4 strip classes
    so zero-padded K tiles never need re-zeroing inside a class). S matmuls
    use kfat [128,128] lhsT = K^T rows at partition strip 32s, zeros
    elsewhere -> every phase A+B matmul runs in the same 128x128 PE tiling
    mode (no PE drain/reconfig). 3-kt S packs -> one [128,1536] PSUM tile ->
    single exp ACT (1.49us, ACT ~100% busy). Emission order S(g+1) before
    PV(g) so the in-order PE queue never stalls on the exp semaphore.
    PV lhsT = V_aug [128,65] (ones col -> softmax denominators free),
    accumulated over 32 key tiles into o_ps [65,512]; drained to bf16
    o1store/o2store (partitions 0-63) + Z rows DMA'd to base-0 z tiles.
  - Phase C tail: wide [16,2048] row math (reciprocal_approx_accurate,
    single Sqrt table switch), sel-matrix ones-matmul row broadcasts,
    mode-batched sub-loops.
  - Phase D proj: bf16 weights (host-cast), K=64 per-head contraction,
    bias added via host-tiled broadcast tensor.
"""

import sys

import numpy as np

for p in ("/opt/trn_rl_repo",):
    if p not in sys.path:
        sys.path.insert(0, p)

import ml_dtypes

import concourse.bass as bass
import concourse.bacc as bacc_mod
import concourse.mybir as mybir
from concourse.bass_utils import run_bass_kernel_spmd
from concourse.tile import TileContext

F32 = mybir.dt.float32
F32R = mybir.dt.float32r
BF16 = mybir.dt.bfloat16

B, N, DIM, H, HD = 4, 4096, 1024, 16, 32
VD = 2 * HD  # 64, per-head v dim
NQ = 2048  # query rows per core
KT = N // 128  # 32 key tiles
CIN = DIM // 128  # 8 contraction tiles
NCORES = 8
LAMBDA_INIT = 0.2
EPS = 1e-5
SCALE = HD ** -0.5

_CACHE = {}


def _r(ap):
    return ap.bitcast(F32R)


def build_nc(lam: float):
    nc = bacc_mod.Bacc(None, target_bir_lowering=False)

    xbt = nc.declare_dram_parameter("xbt", [DIM, N], F32, isOutput=False)
    wqkvt = nc.declare_dram_parameter("wqkvt", [DIM, 3 * DIM], F32, isOutput=False)
    wpbt = nc.declare_dram_parameter("wpbt", [VD, H * DIM], BF16, isOutput=False)
    biasbc = nc.declare_dram_parameter("biasbc", [128, DIM], F32, isOutput=False)
    weff = nc.declare_dram_parameter("weff", [VD, 1], F32, isOutput=False)
    selp = nc.declare_dram_parameter("selp", [H, H * VD], F32, isOutput=False)
    y = nc.declare_dram_parameter("y", [NQ, DIM], F32, isOutput=True)

    qt_s = nc.dram_tensor("qt_scratch", [DIM, NQ], F32)
    o1_s = nc.dram_tensor("o1_scratch", [H * VD, NQ], BF16)
    o2_s = nc.dram_tensor("o2_scratch", [H * VD, NQ], BF16)
    u_s = nc.dram_tensor("u_scratch", [H * VD, NQ], BF16)
    kt_s = nc.dram_tensor("kt_scratch", [DIM, N], F32)
    v_s = nc.dram_tensor("v_scratch", [N, DIM], F32)

    with nc.allow_low_precision(reason="f32r bit-identical fp32; bf16 stores"), \
         TileContext(nc) as tc:
        with (
            tc.tile_pool(name="const", bufs=1) as constp,
            tc.tile_pool(name="store", bufs=1) as storep,
        ):
            ones128 = constp.tile([128, VD], F32R)
            nc.vector.memset(ones128.bitcast(F32), 1.0)
            # sel[:, h, :]: [16, VD] one-hot rows for row->block broadcasts
            sel = constp.tile([H, H, VD], F32R)
            nc.sync.dma_start(
                out=sel,
                in_=selp[:, :].rearrange("p (h v) -> p h v", v=VD)
                .bitcast(F32R))
            weff_t = constp.tile([VD, 1], F32)
            nc.sync.dma_start(out=weff_t, in_=weff[:, :])

            # persistent row stores (base 0)
            z1all = storep.tile([H, NQ], F32)
            z2all = storep.tile([H, NQ], F32)
            msall = storep.tile([H, NQ], F32)

            if True:
                # ================= phase A: qkv =================
                with (
                    tc.tile_pool(name="xbt_p", bufs=2) as xbtp,
                    tc.tile_pool(name="wq_p", bufs=4) as wqp,
                    tc.tile_pool(name="wv_p", bufs=2) as wvp,
                    tc.tile_pool(name="drain_p", bufs=3) as drp,
                    tc.tile_pool(name="psA", bufs=2, space="PSUM") as psA,
                ):
                    for tq in range(4):  # token quarters of 1024
                        xb = xbtp.tile([128, CIN, 1024], F32R, tag="xb")
                        nc.sync.dma_start(
                            out=xb,
                            in_=xbt[:, tq * 1024:(tq + 1) * 1024]
                            .rearrange("(t p) n -> p t n", p=128).bitcast(F32R),
                        )
                        for co in range(2 * CIN):  # 0..7 Q, 8..15 K
                            is_q = co < CIN
                            if is_q and tq >= 2:
                                continue
                            ps = psA.tile([128, 1024], F32, tag="ps")
                            for ci in range(CIN):
                                wt = wqp.tile([128, 128], F32R, tag="w")
                                nc.sync.dma_start(
                                    out=wt,
                                    in_=wqkvt[ci * 128:(ci + 1) * 128,
                                              co * 128:(co + 1) * 128]
                                    .bitcast(F32R),
                                )
                                for sb in range(2):
                                    nc.tensor.matmul(
                                        ps[:, sb * 512:(sb + 1) * 512],
                                        _r(wt),
                                        _r(xb[:, ci, sb * 512:(sb + 1) * 512]),
                                        start=(ci == 0),
                                        stop=(ci == CIN - 1),
                                    )
                            dr = drp.tile([128, 1024], F32, tag="dr")
                            nc.vector.tensor_copy(dr, ps)
                            dst = qt_s if is_q else kt_s
                            coo = co if is_q else co - CIN
                            nc.sync.dma_start(
                                out=dst[coo * 128:(coo + 1) * 128,
                                        tq * 1024:(tq + 1) * 1024],
                                in_=dr,
                            )
                        for cc in range(DIM // 256):
                            wv = wvp.tile([128, CIN, 256], F32R, tag="wv")
                            nc.sync.dma_start(
                                out=wv,
                                in_=wqkvt[:, 2 * DIM + cc * 256:
                                          2 * DIM + (cc + 1) * 256]
                                .rearrange("(t p) n -> p t n", p=128)
                                .bitcast(F32R),
                            )
                            for kt in range(8):
                                psv = psA.tile([128, 256], F32, tag="psv")
                                for ci in range(CIN):
                                    nc.tensor.matmul(
                                        psv,
                                        _r(xb[:, ci, kt * 128:(kt + 1) * 128]),
                                        _r(wv[:, ci, :]),
                                        start=(ci == 0),
                                        stop=(ci == CIN - 1),
                                    )
                                drv = drp.tile([128, 256], F32, tag="drv")
                                nc.vector.tensor_copy(drv, psv)
                                nc.sync.dma_start(
                                    out=v_s[tq * 1024 + kt * 128:
                                            tq * 1024 + (kt + 1) * 128,
                                            cc * 256:(cc + 1) * 256],
                                    in_=drv,
                                )

                # ================= phase B: attention =================
                with (
                    tc.tile_pool(name="kfat_p", bufs=1) as kfatp,
                    tc.tile_pool(name="vh_p", bufs=1) as vhp,
                    tc.tile_pool(name="qp_p", bufs=1) as qpp,
                    tc.tile_pool(name="exp_p", bufs=2) as expp,
                    tc.tile_pool(name="stag_p", bufs=2) as stagp,
                    tc.tile_pool(name="psS", bufs=2, space="PSUM") as psS,
                    tc.tile_pool(name="psO", bufs=2, space="PSUM") as psO,
                ):
                    kfats = [kfatp.tile([128, KT, 128], F32R, tag=f"k{i}", name=f"kfat{i}")
                             for i in range(2)]
                    vhs = [vhp.tile([128, KT, 65], F32R, tag=f"v{i}", name=f"vh{i}")
                           for i in range(2)]
                    qps = [qpp.tile([128, NQ], F32R, tag=f"q{i}", name=f"qp{i}")
                           for i in range(2)]
                    for i in range(2):
                        nc.vector.memset(kfats[i].bitcast(F32), 0.0)
                        nc.vector.memset(vhs[i][:, :, VD:65].bitcast(F32), 1.0)

                    groups = [list(range(3 * g, 3 * g + 3)) for g in range(10)]
                    groups.append([30, 31])
                    packs = []
                    for cidx in range(32):
                        for qb in range(4):
                            for gi, g in enumerate(groups):
                                packs.append(
                                    (cidx, qb, g, gi == 0, gi == len(groups) - 1))

                    st = {"o": {}, "ex": {}}

                    def combo_hbr(idx):
                        s, pc = idx // 8, idx % 8
                        return 2 * pc + (s >> 1), s & 1, s

                    def emit_setup(idx):
                        h, br, s = combo_hbr(idx)
                        kf, vh, qp = kfats[idx % 2], vhs[idx % 2], qps[idx % 2]
                        if idx in (8, 16, 24) and idx % 2 == 0:
                            pass
                        if idx in (8, 16, 24):
                            s_old = (idx - 1) // 8
                            for i in range(2):
                                nc.vector.memset(
                                    kfats[i][32 * s_old:32 * s_old + 32, :, :]
                                    .bitcast(F32), 0.0)
                        r0 = h * VD + br * HD
                        nc.sync.dma_start(
                            out=kf[32 * s:32 * s + 32, :, :],
                            in_=kt_s[r0:r0 + HD, :]
                            .rearrange("p (k t) -> p k t", t=128).bitcast(F32R),
                        )
                        nc.sync.dma_start(
                            out=vh[:, :, 0:VD],
                            in_=v_s[:, h * VD:(h + 1) * VD]
                            .rearrange("(k p) v -> p k v", p=128).bitcast(F32R),
                        )
                        hp = h // 2
                        nc.sync.dma_start(
                            out=qp,
                            in_=qt_s[hp * 128:(hp + 1) * 128, :].bitcast(F32R),
                        )

                    def emit_S(pi):
                        cidx, qb, g, first, last = packs[pi]
                        if first and qb == 0:
                            emit_setup(cidx)
                        kf, qp = kfats[cidx % 2], qps[cidx % 2]
                        n = len(g)
                        sps = psS.tile([128, 1536], F32, tag="s")
                        for i, kt in enumerate(g):
                            nc.tensor.matmul(
                                sps[:, i * 512:(i + 1) * 512],
                                _r(kf[:, kt, :]),
                                _r(qp[:, qb * 512:(qb + 1) * 512]),
                                start=True, stop=True,
                            )
                        ex = expp.tile([128, 1536], F32R, tag="e")
                        nc.scalar.activation(
                            ex[:, 0:n * 512], sps[:, 0:n * 512],
                            mybir.ActivationFunctionType.Exp, scale=SCALE,
                        )
                        st["ex"][pi] = ex

                    def emit_PV(pi):
                        cidx, qb, g, first, last = packs[pi]
                        h, br, s = combo_hbr(cidx)
                        vh = vhs[cidx % 2]
                        ex = st["ex"].pop(pi)
                        if first:
                            st["o"][(cidx, qb)] = psO.tile([65, 512], F32, tag="o", name="o_ps")
                        o_ps = st["o"][(cidx, qb)]
                        for i, kt in enumerate(g):
                            nc.tensor.matmul(
                                o_ps,
                                _r(vh[:, kt, :]),
                                ex[:, i * 512:(i + 1) * 512],
                                start=(kt == 0),
                                stop=(kt == KT - 1),
                            )
                        if last:
                            o_ps = st["o"].pop((cidx, qb))
                            sl = slice(qb * 512, (qb + 1) * 512)
                            stag_o = stagp.tile([VD, 512], BF16, tag="so")
                            nc.vector.tensor_copy(stag_o, o_ps[0:VD, :])
                            odst = o1_s if br == 0 else o2_s
                            nc.sync.dma_start(
                                out=odst[h * VD:(h + 1) * VD, sl], in_=stag_o)
                            stag_z = stagp.tile([65, 512], F32, tag="sz")
                            nc.vector.tensor_copy(
                                stag_z[VD:65, :], o_ps[VD:65, :])
                            zdst = z1all if br == 0 else z2all
                            nc.sync.dma_start(
                                out=zdst[h:h + 1, sl], in_=stag_z[VD:65, :])

                    emit_S(0)
                    for pi in range(len(packs) - 1):
                        emit_S(pi + 1)
                        emit_PV(pi)
                    emit_PV(len(packs) - 1)

                # ============ phase C: tail (combine + norm) ============
                with tc.tile_pool(name="row_p", bufs=1) as rowp:
                    # loop 1: u = o1 - t*o2 -> u_s
                    with (
                        tc.tile_pool(name="rowT", bufs=1) as rowtp,
                        tc.tile_pool(name="inT", bufs=4) as inp,
                        tc.tile_pool(name="scrT", bufs=4) as scrp,
                        tc.tile_pool(name="psT", bufs=2, space="PSUM") as psT,
                    ):
                        rz = rowtp.tile([H, NQ], F32, tag="rz")
                        scr_r = rowtp.tile([H, NQ], F32, tag="scr")
                        nc.vector.reciprocal_approx_accurate(
                            out=rz, in_=z2all, scratch=scr_r)
                        trow = rowtp.tile([H, NQ], F32R, tag="trow")
                        nc.vector.scalar_tensor_tensor(
                            out=trow, in0=z1all, scalar=float(lam), in1=rz,
                            op0=mybir.AluOpType.mult, op1=mybir.AluOpType.mult)
                        for h in range(H):
                            for qb in range(4):
                                sl = slice(qb * 512, (qb + 1) * 512)
                                rsl = slice(h * VD, (h + 1) * VD)
                                o1t = inp.tile([VD, 512], BF16, tag="o1t")
                                nc.sync.dma_start(out=o1t, in_=o1_s[rsl, sl])
                                o2t = inp.tile([VD, 512], BF16, tag="o2t")
                                nc.sync.dma_start(out=o2t, in_=o2_s[rsl, sl])
                                tbc = psT.tile([VD, 512], F32, tag="tbc")
                                nc.tensor.matmul(
                                    tbc, _r(sel[:, h, :]), _r(trow[:, sl]),
                                    start=True, stop=True)
                                o2x = scrp.tile([VD, 512], F32, tag="o2x")
                                nc.vector.tensor_mul(o2x, o2t, tbc)
                                ut = scrp.tile([VD, 512], BF16, tag="ut")
                                nc.vector.tensor_sub(ut, o1t, o2x)
                                nc.sync.dma_start(
                                    out=u_s[rsl, sl], in_=ut)

                    # loop 2: ms = colsum(u^2)
                    with (
                        tc.tile_pool(name="inM", bufs=4) as inp,
                        tc.tile_pool(name="scrM", bufs=4) as scrp,
                        tc.tile_pool(name="psM", bufs=2, space="PSUM") as psM,
                    ):
                        for h in range(H):
                            for qb in range(4):
                                sl = slice(qb * 512, (qb + 1) * 512)
                                rsl = slice(h * VD, (h + 1) * VD)
                                ut = inp.tile([VD, 512], BF16, tag="ut")
                                nc.sync.dma_start(out=ut, in_=u_s[rsl, sl])
                                u2 = scrp.tile([VD, 512], F32R, tag="u2")
                                nc.vector.tensor_mul(u2, ut, ut)
                                ms = psM.tile([1, 512], F32, tag="ms")
                                nc.tensor.matmul(
                                    ms, _r(ones128[0:VD, 0:1]), u2,
                                    start=True, stop=True)
                                msc = scrp.tile([1, 512], F32, tag="msc")
                                nc.vector.tensor_copy(msc, ms)
                                nc.sync.dma_start(
                                    out=msall[h:h + 1, sl], in_=msc)

                    # wide row math + single Sqrt table switch
                    rr0 = rowp.tile([H, NQ], F32R, tag="rr0")
                    with tc.tile_pool(name="rowW", bufs=3) as roww:
                        z1e = roww.tile([H, NQ], F32, tag="t")
                        nc.vector.tensor_scalar_mul(
                            z1e, z1all, float(EPS ** 0.5))
                        z1sq = roww.tile([H, NQ], F32, tag="t")
                        nc.vector.tensor_mul(z1sq, z1e, z1e)
                        arg = roww.tile([H, NQ], F32, tag="t")
                        nc.vector.scalar_tensor_tensor(
                            out=arg, in0=msall, scalar=1.0 / VD, in1=z1sq,
                            op0=mybir.AluOpType.mult, op1=mybir.AluOpType.add)
                        sd = roww.tile([H, NQ], F32, tag="t")
                        nc.scalar.activation(
                            sd, arg, mybir.ActivationFunctionType.Sqrt)
                        rr0f = roww.tile([H, NQ], F32, tag="t")
                        scr2 = roww.tile([H, NQ], F32, tag="t")
                        nc.vector.reciprocal_approx_accurate(
                            out=rr0f, in_=sd, scratch=scr2)
                        nc.vector.tensor_copy(rr0, rr0f)

                    # loop 3: o_n = u * rr * weff -> onstore (bf16)
                    onstore = rowp.tile([VD, H, NQ], BF16, tag="onstore")
                    with (
                        tc.tile_pool(name="inN", bufs=4) as inp,
                        tc.tile_pool(name="scrN", bufs=4) as scrp,
                        tc.tile_pool(name="psN", bufs=2, space="PSUM") as psN,
                    ):
                        for h in range(H):
                            for qb in range(4):
                                sl = slice(qb * 512, (qb + 1) * 512)
                                rsl = slice(h * VD, (h + 1) * VD)
                                ut = inp.tile([VD, 512], BF16, tag="ut")
                                nc.sync.dma_start(out=ut, in_=u_s[rsl, sl])
                                rrbc = psN.tile([VD, 512], F32, tag="rrbc")
                                nc.tensor.matmul(
                                    rrbc, _r(sel[:, h, :]), _r(rr0[:, sl]),
                                    start=True, stop=True)
                                on = scrp.tile([VD, 512], F32, tag="on")
                                nc.vector.tensor_mul(on, ut, rrbc)
                                nc.vector.tensor_scalar_mul(
                                    onstore[:, h, sl], on, weff_t)

                    # ============ phase D: proj (bf16) ============
                    with (
                        tc.tile_pool(name="wp_p", bufs=1) as wpp,
                        tc.tile_pool(name="yd_p", bufs=3) as ydp,
                        tc.tile_pool(name="psY", bufs=2, space="PSUM") as psY,
                    ):
                        wpb = wpp.tile([VD, H, DIM], BF16)
                        nc.sync.dma_start(
                            out=wpb,
                            in_=wpbt[:, :].rearrange("v (h n) -> v h n", h=H))
                        bb = wpp.tile([128, DIM], F32)
                        nc.sync.dma_start(out=bb, in_=biasbc[:, :])
                        for qt in range(NQ // 128):
                            yps = psY.tile([128, 1024], F32, tag="y")
                            for sb in range(2):
                                for h in range(H):
                                    nc.tensor.matmul(
                                        yps[:, sb * 512:(sb + 1) * 512],
                                        onstore[:, h, qt * 128:(qt + 1) * 128],
                                        wpb[:, h, sb * 512:(sb + 1) * 512],
                                        start=(h == 0),
                                        stop=(h == H - 1),
                                    )
                            yd = ydp.tile([128, 1024], F32, tag="yd")
                            nc.vector.tensor_add(yd, yps, bb)
                            nc.sync.dma_start(
                                out=y[qt * 128:(qt + 1) * 128, :], in_=yd)
    nc.finalize()
    return nc


def _make_inputs(x, w_qkv, w_proj, b_proj, sub_norm_w):
    wqkvt = np.ascontiguousarray(np.asarray(w_qkv, np.float32).T)
    wprojt = np.ascontiguousarray(np.asarray(w_proj, np.float32).T)  # [c, out]
    # [VD, H*DIM]: row vd, col h*DIM+out ; source row c = h*VD+vd
    wpbt = np.ascontiguousarray(
        wprojt.reshape(H, VD, DIM).transpose(1, 0, 2).reshape(VD, H * DIM)
    ).astype(ml_dtypes.bfloat16)
    biasbc = np.ascontiguousarray(
        np.tile(np.asarray(b_proj, np.float32).reshape(1, DIM), (128, 1)))
    selp = np.zeros((H, H, VD), np.float32)
    for h in range(H):
        selp[h, h, :] = 1.0
    selp = np.ascontiguousarray(selp.reshape(H, H * VD))
    weff = (np.asarray(sub_norm_w, np.float32)
            * (1.0 - LAMBDA_INIT)).reshape(VD, 1)
    return wqkvt, wpbt, biasbc, weff, selp


def kernel(x, w_qkv, w_proj, b_proj, lambda_q1, lambda_k1, lambda_q2,
           lambda_k2, sub_norm_w):
    x = np.asarray(x, np.float32)
    lam = float(
        np.exp(np.sum(np.float64(lambda_q1) * np.float64(lambda_k1)))
        - np.exp(np.sum(np.float64(lambda_q2) * np.float64(lambda_k2)))
        + LAMBDA_INIT
    )
    wqkvt, wpbt, biasbc, weff, selp = _make_inputs(
        x, w_qkv, w_proj, b_proj, sub_norm_w)

    key = round(lam, 12)
    if key not in _CACHE:
        _CACHE[key] = build_nc(lam)
    nc = _CACHE[key]

    in_maps = []
    for c in range(NCORES):
        b, half = c // 2, c % 2
        xt = np.asarray(x[b].T)  # [DIM, N]
        if half == 1:  # query rows first
            xt = np.concatenate([xt[:, NQ:], xt[:, :NQ]], axis=1)
        in_maps.append({
            "xbt": np.ascontiguousarray(xt),
            "wqkvt": wqkvt,
            "wpbt": wpbt,
            "biasbc": biasbc,
            "weff": weff,
            "selp": selp,
        })
    res = run_bass_kernel_spmd(nc, in_maps, list(range(NCORES)))
    out = np.empty((B, N, DIM), np.float32)
    for c in range(NCORES):
        b, half = c // 2, c % 2
        out[b, half * NQ:(half + 1) * NQ, :] = res.results[c]["y"]
    return out


# revision 18
# speedup vs baseline: 1.9786x; 1.0273x over previous
"""DiffAttention TRN2 kernel: 8-way (batch x seq-half) sharded, zero collectives.

v2 pipeline (ACT-exp is the roofline: 268M exps/core ~= 2.05ms):
  - Phase A: qkv projections to DRAM scratch (Q^T, K^T, V), token-quartered.
  - Phase B attention, combo-major ((head,branch) = combo, in 4 strip classes
    so zero-padded K tiles never need re-zeroing inside a class). S matmuls
    use kfat [128,128] lhsT = K^T rows at partition strip 32s, zeros
    elsewhere -> every phase A+B matmul runs in the same 128x128 PE tiling
    mode (no PE drain/reconfig). 3-kt S packs -> one [128,1536] PSUM tile ->
    single exp ACT (1.49us, ACT ~100% busy). Emission order S(g+1) before
    PV(g) so the in-order PE queue never stalls on the exp semaphore.
    PV lhsT = V_aug [128,65] (ones col -> softmax denominators free),
    accumulated over 32 key tiles into o_ps [65,512]; drained to bf16
    o1store/o2store (partitions 0-63) + Z rows DMA'd to base-0 z tiles.
  - Phase C tail: wide [16,2048] row math (reciprocal_approx_accurate,
    single Sqrt table switch), sel-matrix ones-matmul row broadcasts,
    mode-batched sub-loops.
  - Phase D proj: bf16 weights (host-cast), K=64 per-head contraction,
    bias added via host-tiled broadcast tensor.
"""

import sys

import numpy as np

for p in ("/opt/trn_rl_repo",):
    if p not in sys.path:
        sys.path.insert(0, p)

import ml_dtypes

import concourse.bass as bass
import concourse.bacc as bacc_mod
import concourse.mybir as mybir
from concourse.bass_utils import run_bass_kernel_spmd
from concourse.tile import TileContext

F32 = mybir.dt.float32
F32R = mybir.dt.float32r
BF16 = mybir.dt.bfloat16

B, N, DIM, H, HD = 4, 4096, 1024, 16, 32
VD = 2 * HD  # 64, per-head v dim
NQ = 2048  # query rows per core
KT = N // 128  # 32 key tiles
CIN = DIM // 128  # 8 contraction tiles
NCORES = 8
LAMBDA_INIT = 0.2
EPS = 1e-5
SCALE = HD ** -0.5

_CACHE = {}


def _r(ap):
    return ap.bitcast(F32R)


def build_nc(lam: float):
    nc = bacc_mod.Bacc(None, target_bir_lowering=False)

    xbt = nc.declare_dram_parameter("xbt", [DIM, N], F32, isOutput=False)
    wqkvt = nc.declare_dram_parameter("wqkvt", [DIM, 3 * DIM], F32, isOutput=False)
    wpbt = nc.declare_dram_parameter("wpbt", [128, CIN * DIM], BF16, isOutput=False)
    biasbc = nc.declare_dram_parameter("biasbc", [128, DIM], F32, isOutput=False)
    weff = nc.declare_dram_parameter("weff", [VD, 1], F32, isOutput=False)
    selp = nc.declare_dram_parameter("selp", [128, H * 128], F32, isOutput=False)
    y = nc.declare_dram_parameter("y", [NQ, DIM], F32, isOutput=True)

    qt_s = nc.dram_tensor("qt_scratch", [DIM, NQ], F32)
    o1_s = nc.dram_tensor("o1_scratch", [H * VD, NQ], BF16)
    u_s = nc.dram_tensor("u_scratch", [H * VD, NQ], BF16)
    kt_s = nc.dram_tensor("kt_scratch", [DIM, N], F32)
    v_s = nc.dram_tensor("v_scratch", [N, DIM], F32)

    with nc.allow_low_precision(reason="f32r bit-identical fp32; bf16 stores"), \
         TileContext(nc) as tc:
        with (
            tc.tile_pool(name="const", bufs=1) as constp,
            tc.tile_pool(name="store", bufs=1) as storep,
        ):
            # padrow0: row 0 = [1]*VD pad 0 -> broadcast partition-0 row
            padrow0 = constp.tile([128, 128], F32R)
            nc.vector.memset(padrow0.bitcast(F32), 0.0)
            nc.vector.memset(padrow0[0:1, 0:VD].bitcast(F32), 1.0)
            # padcol: col 0 = ones on partitions 0-63 -> colsum over vd
            padcol = constp.tile([128, 128], F32R)
            nc.vector.memset(padcol.bitcast(F32), 0.0)
            nc.vector.memset(padcol[0:VD, 0:1].bitcast(F32), 1.0)
            # selfat[:, h, :]: [128, 128] one-hot padded broadcast matrices
            selfat = constp.tile([128, H, 128], F32R)
            nc.sync.dma_start(
                out=selfat,
                in_=selp[:, :].rearrange("p (h v) -> p h v", v=128)
                .bitcast(F32R))
            weff_t = constp.tile([VD, 1], F32)
            nc.sync.dma_start(out=weff_t, in_=weff[:, :])

            # persistent row stores (base 0)
            z1all = storep.tile([H, NQ], F32)
            argall = storep.tile([H, NQ], F32)

            if True:
                # ================= phase A: qkv =================
                with (
                    tc.tile_pool(name="xbt_p", bufs=2) as xbtp,
                    tc.tile_pool(name="wq_p", bufs=4) as wqp,
                    tc.tile_pool(name="wv_p", bufs=2) as wvp,
                    tc.tile_pool(name="drain_p", bufs=3) as drp,
                    tc.tile_pool(name="psA", bufs=3, space="PSUM") as psA,
                    tc.tile_pool(name="psAv", bufs=2, space="PSUM") as psAv,
                ):
                    for tq in range(4):  # token quarters of 1024
                        xb = xbtp.tile([128, CIN, 1024], F32R, tag="xb")
                        nc.sync.dma_start(
                            out=xb,
                            in_=xbt[:, tq * 1024:(tq + 1) * 1024]
                            .rearrange("(t p) n -> p t n", p=128).bitcast(F32R),
                        )
                        for co in range(2 * CIN):  # 0..7 Q, 8..15 K
                            is_q = co < CIN
                            if is_q and tq >= 2:
                                continue
                            ps = psA.tile([128, 1024], F32, tag="ps")
                            for ci in range(CIN):
                                wt = wqp.tile([128, 128], F32R, tag="w")
                                nc.sync.dma_start(
                                    out=wt,
                                    in_=wqkvt[ci * 128:(ci + 1) * 128,
                                              co * 128:(co + 1) * 128]
                                    .bitcast(F32R),
                                )
                                for sb in range(2):
                                    nc.tensor.matmul(
                                        ps[:, sb * 512:(sb + 1) * 512],
                                        _r(wt),
                                        _r(xb[:, ci, sb * 512:(sb + 1) * 512]),
                                        start=(ci == 0),
                                        stop=(ci == CIN - 1),
                                    )
                            dr = drp.tile([128, 1024], F32, tag="dr")
                            nc.vector.tensor_copy(dr, ps)
                            dst = qt_s if is_q else kt_s
                            coo = co if is_q else co - CIN
                            nc.sync.dma_start(
                                out=dst[coo * 128:(coo + 1) * 128,
                                        tq * 1024:(tq + 1) * 1024],
                                in_=dr,
                            )
                        for cc in range(DIM // 256):
                            wv = wvp.tile([128, CIN, 256], F32R, tag="wv")
                            nc.sync.dma_start(
                                out=wv,
                                in_=wqkvt[:, 2 * DIM + cc * 256:
                                          2 * DIM + (cc + 1) * 256]
                                .rearrange("(t p) n -> p t n", p=128)
                                .bitcast(F32R),
                            )
                            for kt in range(8):
                                psv = psAv.tile([128, 256], F32, tag="psv")
                                for ci in range(CIN):
                                    nc.tensor.matmul(
                                        psv,
                                        _r(xb[:, ci, kt * 128:(kt + 1) * 128]),
                                        _r(wv[:, ci, :]),
                                        start=(ci == 0),
                                        stop=(ci == CIN - 1),
                                    )
                                drv = drp.tile([128, 256], F32, tag="drv")
                                if kt % 2 == 0:
                                    nc.vector.tensor_copy(drv, psv)
                                else:
                                    nc.scalar.activation(
                                        drv, psv,
                                        mybir.ActivationFunctionType.Copy)
                                nc.sync.dma_start(
                                    out=v_s[tq * 1024 + kt * 128:
                                            tq * 1024 + (kt + 1) * 128,
                                            cc * 256:(cc + 1) * 256],
                                    in_=drv,
                                )

                # ================= phase B: attention =================
                with (
                    tc.tile_pool(name="kfat_p", bufs=1) as kfatp,
                    tc.tile_pool(name="vh_p", bufs=1) as vhp,
                    tc.tile_pool(name="qp_p", bufs=1) as qpp,
                    tc.tile_pool(name="exp_p", bufs=3) as expp,
                    tc.tile_pool(name="stag_p", bufs=2) as stagp,
                    tc.tile_pool(name="hsc_p", bufs=2) as hscp,
                    tc.tile_pool(name="psS", bufs=2, space="PSUM") as psS,
                    tc.tile_pool(name="psO", bufs=2, space="PSUM") as psO,
                ):
                    kfats = [kfatp.tile([128, KT, 128], F32R, tag=f"k{i}", name=f"kfat{i}")
                             for i in range(2)]
                    vhs = [vhp.tile([128, KT, 65], F32R, tag=f"v{i}", name=f"vh{i}")
                           for i in range(2)]
                    qps = [qpp.tile([128, NQ], F32R, tag=f"q{i}", name=f"qp{i}")
                           for i in range(2)]
                    for i in range(2):
                        nc.vector.memset(kfats[i].bitcast(F32), 0.0)
                        nc.vector.memset(vhs[i][:, :, VD:65].bitcast(F32), 1.0)

                    groups = [list(range(3 * g, 3 * g + 3)) for g in range(10)]
                    groups.append([30, 31])
                    packs = []
                    for cidx in range(32):
                        for qb in range(4):
                            for gi, g in enumerate(groups):
                                packs.append(
                                    (cidx, qb, g, gi == 0, gi == len(groups) - 1))

                    st = {"o": {}, "ex": {}}

                    def combo_hbr(idx):
                        s, pc = idx // 8, idx % 8
                        return 2 * pc + (s >> 1), s & 1, s

                    def emit_setup(idx):
                        h, br, s = combo_hbr(idx)
                        kf, vh, qp = kfats[idx % 2], vhs[idx % 2], qps[idx % 2]
                        if idx in (8, 16, 24) and idx % 2 == 0:
                            pass
                        if idx in (8, 16, 24):
                            s_old = (idx - 1) // 8
                            for i in range(2):
                                nc.vector.memset(
                                    kfats[i][32 * s_old:32 * s_old + 32, :, :]
                                    .bitcast(F32), 0.0)
                        r0 = h * VD + br * HD
                        nc.sync.dma_start(
                            out=kf[32 * s:32 * s + 32, :, :],
                            in_=kt_s[r0:r0 + HD, :]
                            .rearrange("p (k t) -> p k t", t=128).bitcast(F32R),
                        )
                        nc.sync.dma_start(
                            out=vh[:, :, 0:VD],
                            in_=v_s[:, h * VD:(h + 1) * VD]
                            .rearrange("(k p) v -> p k v", p=128).bitcast(F32R),
                        )
                        hp = h // 2
                        nc.sync.dma_start(
                            out=qp,
                            in_=qt_s[hp * 128:(hp + 1) * 128, :].bitcast(F32R),
                        )

                    def emit_S(pi):
                        cidx, qb, g, first, last = packs[pi]
                        if first and qb == 0:
                            emit_setup(cidx)
                        kf, qp = kfats[cidx % 2], qps[cidx % 2]
                        n = len(g)
                        sps = psS.tile([128, 1536], F32, tag="s")
                        for i, kt in enumerate(g):
                            nc.tensor.matmul(
                                sps[:, i * 512:(i + 1) * 512],
                                _r(kf[:, kt, :]),
                                _r(qp[:, qb * 512:(qb + 1) * 512]),
                                start=True, stop=True,
                            )
                        ex = expp.tile([128, 1536], F32R, tag="e")
                        nc.scalar.activation(
                            ex[:, 0:n * 512], sps[:, 0:n * 512],
                            mybir.ActivationFunctionType.Exp, scale=SCALE,
                        )
                        st["ex"][pi] = ex

                    def emit_PV(pi):
                        cidx, qb, g, first, last = packs[pi]
                        h, br, s = combo_hbr(cidx)
                        vh = vhs[cidx % 2]
                        ex = st["ex"].pop(pi)
                        if first:
                            st["o"][(cidx, qb)] = psO.tile([65, 512], F32, tag="o", name="o_ps")
                        o_ps = st["o"][(cidx, qb)]
                        for i, kt in enumerate(g):
                            nc.tensor.matmul(
                                o_ps,
                                _r(vh[:, kt, :]),
                                ex[:, i * 512:(i + 1) * 512],
                                start=(kt == 0),
                                stop=(kt == KT - 1),
                            )
                        if last and br == 0:
                            o_ps = st["o"].pop((cidx, qb))
                            sl = slice(qb * 512, (qb + 1) * 512)
                            stag_o = stagp.tile([VD, 512], BF16, tag="so")
                            nc.vector.tensor_copy(stag_o, o_ps[0:VD, :])
                            nc.sync.dma_start(
                                out=o1_s[h * VD:(h + 1) * VD, sl], in_=stag_o)
                            stag_z = stagp.tile([65, 512], F32, tag="sz")
                            nc.vector.tensor_copy(
                                stag_z[VD:65, :], o_ps[VD:65, :])
                            nc.sync.dma_start(
                                out=z1all[h:h + 1, sl], in_=stag_z[VD:65, :])
                        elif last:
                            # br1: hoisted combine u = o1 - (lam*Z1/Z2)*o2,
                            # arg = mean(u^2) + eps*Z1^2  (all 128-mode MMs)
                            o_ps = st["o"].pop((cidx, qb))
                            sl = slice(qb * 512, (qb + 1) * 512)
                            rsl = slice(h * VD, (h + 1) * VD)
                            zp0 = hscp.tile([1, 512], F32, tag="zp0")
                            nc.sync.dma_start(
                                out=zp0, in_=z1all[h:h + 1, sl])
                            o1t = hscp.tile([VD, 512], BF16, tag="o1t")
                            nc.sync.dma_start(out=o1t, in_=o1_s[rsl, sl])
                            z2c = hscp.tile([65, 512], F32, tag="z2c")
                            nc.vector.tensor_copy(
                                z2c[VD:65, :], o_ps[VD:65, :])
                            z2p0 = hscp.tile([1, 512], F32, tag="z2p0")
                            nc.sync.dma_start(
                                out=z2p0, in_=z2c[VD:65, :])
                            rz2 = hscp.tile([1, 512], F32, tag="rz2")
                            rzs = hscp.tile([1, 512], F32, tag="rzs")
                            nc.vector.reciprocal_approx_accurate(
                                out=rz2, in_=z2p0, scratch=rzs)
                            trowf = hscp.tile([128, 512], F32R, tag="trowf")
                            nc.vector.memset(trowf.bitcast(F32), 0.0)
                            nc.vector.scalar_tensor_tensor(
                                out=trowf[0:1, :], in0=zp0,
                                scalar=float(lam), in1=rz2,
                                op0=mybir.AluOpType.mult,
                                op1=mybir.AluOpType.mult)
                            tbc = psO.tile([128, 512], F32, tag="o",
                                           name="tbc")
                            nc.tensor.matmul(
                                tbc, padrow0, trowf, start=True, stop=True)
                            o2s = hscp.tile([VD, 512], F32, tag="o2s")
                            nc.vector.tensor_copy(o2s, o_ps[0:VD, :])
                            o2x = hscp.tile([VD, 512], F32, tag="o2x")
                            nc.vector.tensor_mul(o2x, o2s, tbc[0:VD, :])
                            ut = hscp.tile([VD, 512], BF16, tag="ut")
                            nc.vector.tensor_sub(ut, o1t, o2x)
                            nc.sync.dma_start(out=u_s[rsl, sl], in_=ut)
                            u2f = hscp.tile([128, 512], F32R, tag="u2f")
                            nc.vector.memset(
                                u2f[VD:128, :].bitcast(F32), 0.0)
                            nc.vector.tensor_mul(u2f[0:VD, :], ut, ut)
                            mps = psO.tile([128, 512], F32, tag="o",
                                           name="mps")
                            nc.tensor.matmul(
                                mps, padcol, u2f, start=True, stop=True)
                            ze = hscp.tile([1, 512], F32, tag="ze")
                            nc.vector.tensor_scalar_mul(
                                ze, zp0, float(EPS ** 0.5))
                            zsq = hscp.tile([1, 512], F32, tag="zsq")
                            nc.vector.tensor_mul(zsq, ze, ze)
                            arg0 = hscp.tile([1, 512], F32, tag="arg0")
                            nc.vector.scalar_tensor_tensor(
                                out=arg0, in0=mps[0:1, :], scalar=1.0 / VD,
                                in1=zsq,
                                op0=mybir.AluOpType.mult,
                                op1=mybir.AluOpType.add)
                            nc.sync.dma_start(
                                out=argall[h:h + 1, sl], in_=arg0)

                    emit_S(0)
                    for pi in range(len(packs) - 1):
                        emit_S(pi + 1)
                        emit_PV(pi)
                    emit_PV(len(packs) - 1)

                # ============ phase C: tail (norm + proj) ============
                with tc.tile_pool(name="row_p", bufs=1) as rowp:
                    rr0f = rowp.tile([128, NQ], F32R, tag="rr0f")
                    nc.vector.memset(rr0f.bitcast(F32), 0.0)
                    with tc.tile_pool(name="rowW", bufs=3) as roww:
                        sd = roww.tile([H, NQ], F32, tag="t")
                        nc.scalar.activation(
                            sd, argall, mybir.ActivationFunctionType.Sqrt)
                        rrt = roww.tile([H, NQ], F32, tag="t")
                        scr2 = roww.tile([H, NQ], F32, tag="t")
                        nc.vector.reciprocal_approx_accurate(
                            out=rrt, in_=sd, scratch=scr2)
                        nc.vector.tensor_copy(rr0f[0:H, :], rrt)

                    # o_n = u * rr * weff -> onstore [128, CIN, NQ] bf16
                    onstore = rowp.tile([128, CIN, NQ], BF16, tag="onstore")
                    with (
                        tc.tile_pool(name="inN", bufs=4) as inp,
                        tc.tile_pool(name="scrN", bufs=4) as scrp,
                        tc.tile_pool(name="psN", bufs=2, space="PSUM") as psN,
                    ):
                        for h in range(H):
                            for qb in range(4):
                                sl = slice(qb * 512, (qb + 1) * 512)
                                rsl = slice(h * VD, (h + 1) * VD)
                                ut = inp.tile([VD, 512], BF16, tag="ut")
                                nc.sync.dma_start(out=ut, in_=u_s[rsl, sl])
                                rrbc = psN.tile([128, 512], F32, tag="rrbc")
                                nc.tensor.matmul(
                                    rrbc, selfat[:, h, :], rr0f[:, sl],
                                    start=True, stop=True)
                                on = scrp.tile([VD, 512], F32, tag="on")
                                nc.vector.tensor_mul(on, ut, rrbc[0:VD, :])
                                if h % 2 == 0:
                                    nc.vector.tensor_scalar_mul(
                                        onstore[0:VD, h // 2, sl], on, weff_t)
                                else:
                                    onb = scrp.tile([VD, 512], BF16, tag="onb")
                                    nc.vector.tensor_scalar_mul(
                                        onb, on, weff_t)
                                    nc.sync.dma_start(
                                        out=onstore[VD:128, h // 2, sl],
                                        in_=onb)

                    # ============ phase D: proj (bf16, K=128) ============
                    with (
                        tc.tile_pool(name="wp_p", bufs=1) as wpp,
                        tc.tile_pool(name="yd_p", bufs=3) as ydp,
                        tc.tile_pool(name="psY", bufs=2, space="PSUM") as psY,
                    ):
                        wpb = wpp.tile([128, CIN, DIM], BF16)
                        nc.sync.dma_start(
                            out=wpb,
                            in_=wpbt[:, :].rearrange("v (c n) -> v c n", c=CIN))
                        bb = wpp.tile([128, DIM], F32)
                        nc.sync.dma_start(out=bb, in_=biasbc[:, :])
                        for qt in range(NQ // 128):
                            yps = psY.tile([128, 1024], F32, tag="y")
                            for sb in range(2):
                                for ci in range(CIN):
                                    nc.tensor.matmul(
                                        yps[:, sb * 512:(sb + 1) * 512],
                                        onstore[:, ci, qt * 128:(qt + 1) * 128],
                                        wpb[:, ci, sb * 512:(sb + 1) * 512],
                                        start=(ci == 0),
                                        stop=(ci == CIN - 1),
                                    )
                            yd = ydp.tile([128, 1024], F32, tag="yd")
                            nc.vector.tensor_add(yd, yps, bb)
                            nc.sync.dma_start(
                                out=y[qt * 128:(qt + 1) * 128, :], in_=yd)
    nc.finalize()
    return nc


def _make_inputs(x, w_qkv, w_proj, b_proj, sub_norm_w):
    wqkvt = np.ascontiguousarray(np.asarray(w_qkv, np.float32).T)
    wprojt = np.ascontiguousarray(np.asarray(w_proj, np.float32).T)  # [c, out]
    # proj weights: partition (h%2)*64+vd, col (h//2)*DIM+out
    wpbt = np.ascontiguousarray(
        wprojt.reshape(CIN, 2, VD, DIM).transpose(1, 2, 0, 3)
        .reshape(128, CIN * DIM)).astype(ml_dtypes.bfloat16)
    biasbc = np.ascontiguousarray(
        np.tile(np.asarray(b_proj, np.float32).reshape(1, DIM), (128, 1)))
    # selfat[:, h, :]: [128,128]; rows 0-15 hold one-hot h -> cols 0-63
    selp = np.zeros((128, H, 128), np.float32)
    for h in range(H):
        selp[h, h, 0:VD] = 1.0
    selp = np.ascontiguousarray(selp.reshape(128, H * 128))
    weff = (np.asarray(sub_norm_w, np.float32)
            * (1.0 - LAMBDA_INIT)).reshape(VD, 1)
    return wqkvt, wpbt, biasbc, weff, selp


def kernel(x, w_qkv, w_proj, b_proj, lambda_q1, lambda_k1, lambda_q2,
           lambda_k2, sub_norm_w):
    x = np.asarray(x, np.float32)
    lam = float(
        np.exp(np.sum(np.float64(lambda_q1) * np.float64(lambda_k1)))
        - np.exp(np.sum(np.float64(lambda_q2) * np.float64(lambda_k2)))
        + LAMBDA_INIT
    )
    wqkvt, wpbt, biasbc, weff, selp = _make_inputs(
        x, w_qkv, w_proj, b_proj, sub_norm_w)

    key = round(lam, 12)
    if key not in _CACHE:
        _CACHE[key] = build_nc(lam)
    nc = _CACHE[key]

    in_maps = []
    for c in range(NCORES):
        b, half = c // 2, c % 2
        xt = np.asarray(x[b].T)  # [DIM, N]
        if half == 1:  # query rows first
            xt = np.concatenate([xt[:, NQ:], xt[:, :NQ]], axis=1)
        in_maps.append({
            "xbt": np.ascontiguousarray(xt),
            "wqkvt": wqkvt,
            "wpbt": wpbt,
            "biasbc": biasbc,
            "weff": weff,
            "selp": selp,
        })
    res = run_bass_kernel_spmd(nc, in_maps, list(range(NCORES)))
    out = np.empty((B, N, DIM), np.float32)
    for c in range(NCORES):
        b, half = c // 2, c % 2
        out[b, half * NQ:(half + 1) * NQ, :] = res.results[c]["y"]
    return out


# revision 19
# speedup vs baseline: 2.1686x; 1.0960x over previous
"""DiffAttention TRN2 kernel: 8-way (batch x seq-half) sharded, zero collectives.

v2 pipeline (ACT-exp is the roofline: 268M exps/core ~= 2.05ms):
  - Phase A: qkv projections to DRAM scratch (Q^T, K^T, V), token-quartered.
  - Phase B attention, combo-major ((head,branch) = combo, in 4 strip classes
    so zero-padded K tiles never need re-zeroing inside a class). S matmuls
    use kfat [128,128] lhsT = K^T rows at partition strip 32s, zeros
    elsewhere -> every phase A+B matmul runs in the same 128x128 PE tiling
    mode (no PE drain/reconfig). 3-kt S packs -> one [128,1536] PSUM tile ->
    single exp ACT (1.49us, ACT ~100% busy). Emission order S(g+1) before
    PV(g) so the in-order PE queue never stalls on the exp semaphore.
    PV lhsT = V_aug [128,65] (ones col -> softmax denominators free),
    accumulated over 32 key tiles into o_ps [65,512]; drained to bf16
    o1store/o2store (partitions 0-63) + Z rows DMA'd to base-0 z tiles.
  - Phase C tail: wide [16,2048] row math (reciprocal_approx_accurate,
    single Sqrt table switch), sel-matrix ones-matmul row broadcasts,
    mode-batched sub-loops.
  - Phase D proj: bf16 weights (host-cast), K=64 per-head contraction,
    bias added via host-tiled broadcast tensor.
"""

import sys

import numpy as np

for p in ("/opt/trn_rl_repo",):
    if p not in sys.path:
        sys.path.insert(0, p)

import ml_dtypes

import concourse.bass as bass
import concourse.bacc as bacc_mod
import concourse.mybir as mybir
from concourse.bass_utils import run_bass_kernel_spmd
from concourse.tile import TileContext

F32 = mybir.dt.float32
F32R = mybir.dt.float32r
BF16 = mybir.dt.bfloat16

B, N, DIM, H, HD = 4, 4096, 1024, 16, 32
VD = 2 * HD  # 64, per-head v dim
NQ = 2048  # query rows per core
KT = N // 128  # 32 key tiles
CIN = DIM // 128  # 8 contraction tiles
NCORES = 8
LAMBDA_INIT = 0.2
EPS = 1e-5
SCALE = HD ** -0.5

_CACHE = {}


def _r(ap):
    return ap.bitcast(F32R)


def build_nc(lam: float):
    nc = bacc_mod.Bacc(None, target_bir_lowering=False)

    xbt = nc.declare_dram_parameter("xbt", [DIM, N], F32, isOutput=False)
    wqkvt = nc.declare_dram_parameter("wqkvt", [DIM, 3 * DIM], F32, isOutput=False)
    wpbt = nc.declare_dram_parameter("wpbt", [128, CIN * DIM], BF16, isOutput=False)
    biasbc = nc.declare_dram_parameter("biasbc", [128, DIM], F32, isOutput=False)
    weff = nc.declare_dram_parameter("weff", [VD, 1], F32, isOutput=False)
    selp = nc.declare_dram_parameter("selp", [128, H * 128], F32, isOutput=False)
    y = nc.declare_dram_parameter("y", [NQ, DIM], F32, isOutput=True)

    qt_s = nc.dram_tensor("qt_scratch", [DIM, NQ], F32)
    o1_s = nc.dram_tensor("o1_scratch", [H * VD, NQ], BF16)
    u_s = nc.dram_tensor("u_scratch", [H * VD, NQ], BF16)
    kt_s = nc.dram_tensor("kt_scratch", [DIM, N], F32)
    v_s = nc.dram_tensor("v_scratch", [N, DIM], F32)

    with nc.allow_low_precision(reason="f32r bit-identical fp32; bf16 stores"), \
         TileContext(nc) as tc:
        with (
            tc.tile_pool(name="const", bufs=1) as constp,
            tc.tile_pool(name="store", bufs=1) as storep,
        ):
            # padrow0: row 0 = [1]*VD pad 0 -> broadcast partition-0 row
            padrow0 = constp.tile([128, 128], F32R)
            nc.vector.memset(padrow0.bitcast(F32), 0.0)
            nc.vector.memset(padrow0[0:1, 0:VD].bitcast(F32), 1.0)
            # padcol: col 0 = ones on partitions 0-63 -> colsum over vd
            padcol = constp.tile([128, 128], F32R)
            nc.vector.memset(padcol.bitcast(F32), 0.0)
            nc.vector.memset(padcol[0:VD, 0:1].bitcast(F32), 1.0)
            # selfat[:, h, :]: [128, 128] one-hot padded broadcast matrices
            selfat = constp.tile([128, H, 128], F32R)
            nc.sync.dma_start(
                out=selfat,
                in_=selp[:, :].rearrange("p (h v) -> p h v", v=128)
                .bitcast(F32R))
            weff_t = constp.tile([VD, 1], F32)
            nc.sync.dma_start(out=weff_t, in_=weff[:, :])

            # persistent row stores (base 0)
            z1all = storep.tile([H, NQ], F32)
            argall = storep.tile([H, NQ], F32)

            if True:
                # ================= phase A: qkv =================
                with (
                    tc.tile_pool(name="xbt_p", bufs=2) as xbtp,
                    tc.tile_pool(name="wq_p", bufs=4) as wqp,
                    tc.tile_pool(name="wv_p", bufs=2) as wvp,
                    tc.tile_pool(name="drain_p", bufs=3) as drp,
                    tc.tile_pool(name="psA", bufs=3, space="PSUM") as psA,
                    tc.tile_pool(name="psAv", bufs=2, space="PSUM") as psAv,
                ):
                    for tq in range(4):  # token quarters of 1024
                        xb = xbtp.tile([128, CIN, 1024], F32R, tag="xb")
                        nc.sync.dma_start(
                            out=xb,
                            in_=xbt[:, tq * 1024:(tq + 1) * 1024]
                            .rearrange("(t p) n -> p t n", p=128).bitcast(F32R),
                        )
                        for co in range(2 * CIN):  # 0..7 Q, 8..15 K
                            is_q = co < CIN
                            if is_q and tq >= 2:
                                continue
                            ps = psA.tile([128, 1024], F32, tag="ps")
                            for ci in range(CIN):
                                wt = wqp.tile([128, 128], F32R, tag="w")
                                nc.sync.dma_start(
                                    out=wt,
                                    in_=wqkvt[ci * 128:(ci + 1) * 128,
                                              co * 128:(co + 1) * 128]
                                    .bitcast(F32R),
                                )
                                for sb in range(2):
                                    nc.tensor.matmul(
                                        ps[:, sb * 512:(sb + 1) * 512],
                                        _r(wt),
                                        _r(xb[:, ci, sb * 512:(sb + 1) * 512]),
                                        start=(ci == 0),
                                        stop=(ci == CIN - 1),
                                    )
                            dr = drp.tile([128, 1024], F32, tag="dr")
                            nc.vector.tensor_copy(dr, ps)
                            dst = qt_s if is_q else kt_s
                            coo = co if is_q else co - CIN
                            nc.sync.dma_start(
                                out=dst[coo * 128:(coo + 1) * 128,
                                        tq * 1024:(tq + 1) * 1024],
                                in_=dr,
                            )
                        for cc in range(DIM // 256):
                            wv = wvp.tile([128, CIN, 256], F32R, tag="wv")
                            nc.sync.dma_start(
                                out=wv,
                                in_=wqkvt[:, 2 * DIM + cc * 256:
                                          2 * DIM + (cc + 1) * 256]
                                .rearrange("(t p) n -> p t n", p=128)
                                .bitcast(F32R),
                            )
                            for kt in range(8):
                                psv = psAv.tile([128, 256], F32, tag="psv")
                                for ci in range(CIN):
                                    nc.tensor.matmul(
                                        psv,
                                        _r(xb[:, ci, kt * 128:(kt + 1) * 128]),
                                        _r(wv[:, ci, :]),
                                        start=(ci == 0),
                                        stop=(ci == CIN - 1),
                                    )
                                drv = drp.tile([128, 256], F32, tag="drv")
                                if kt % 2 == 0:
                                    nc.vector.tensor_copy(drv, psv)
                                else:
                                    nc.scalar.activation(
                                        drv, psv,
                                        mybir.ActivationFunctionType.Copy)
                                nc.sync.dma_start(
                                    out=v_s[tq * 1024 + kt * 128:
                                            tq * 1024 + (kt + 1) * 128,
                                            cc * 256:(cc + 1) * 256],
                                    in_=drv,
                                )

                # ================= phase B: attention =================
                with (
                    tc.tile_pool(name="kfat_p", bufs=1) as kfatp,
                    tc.tile_pool(name="vh_p", bufs=1) as vhp,
                    tc.tile_pool(name="qp_p", bufs=1) as qpp,
                    tc.tile_pool(name="exp_p", bufs=3) as expp,
                    tc.tile_pool(name="stag_p", bufs=2) as stagp,
                    tc.tile_pool(name="hsc_p", bufs=2) as hscp,
                    tc.tile_pool(name="psS", bufs=2, space="PSUM") as psS,
                    tc.tile_pool(name="psO", bufs=2, space="PSUM") as psO,
                ):
                    kfats = [kfatp.tile([128, KT, 128], F32R, tag=f"k{i}", name=f"kfat{i}")
                             for i in range(2)]
                    vhs = [vhp.tile([128, KT, 65], F32R, tag=f"v{i}", name=f"vh{i}")
                           for i in range(2)]
                    qps = [qpp.tile([128, NQ], F32R, tag=f"q{i}", name=f"qp{i}")
                           for i in range(2)]
                    trowfs = [hscp.tile([128, 512], F32R, tag=f"tr{i}",
                                        name=f"trowf{i}") for i in range(2)]
                    u2fs = [hscp.tile([128, 512], F32R, tag=f"u2{i}",
                                      name=f"u2f{i}") for i in range(2)]
                    for i in range(2):
                        nc.vector.memset(trowfs[i].bitcast(F32), 0.0)
                        nc.vector.memset(u2fs[i][VD:128, :].bitcast(F32), 0.0)
                    for i in range(2):
                        nc.vector.memset(kfats[i].bitcast(F32), 0.0)
                        nc.vector.memset(vhs[i][:, :, VD:65].bitcast(F32), 1.0)

                    groups = [list(range(3 * g, 3 * g + 3)) for g in range(10)]
                    groups.append([30, 31])
                    packs = []
                    for cidx in range(32):
                        for qb in range(4):
                            for gi, g in enumerate(groups):
                                packs.append(
                                    (cidx, qb, g, gi == 0, gi == len(groups) - 1))

                    st = {"o": {}, "ex": {}}
                    sched = {}

                    def combo_hbr(idx):
                        s, pc = idx // 8, idx % 8
                        return 2 * pc + (s >> 1), s & 1, s

                    def emit_setup(idx):
                        h, br, s = combo_hbr(idx)
                        kf, vh, qp = kfats[idx % 2], vhs[idx % 2], qps[idx % 2]
                        if idx in (8, 16, 24) and idx % 2 == 0:
                            pass
                        if idx in (8, 16, 24):
                            s_old = (idx - 1) // 8
                            for i in range(2):
                                nc.vector.memset(
                                    kfats[i][32 * s_old:32 * s_old + 32, :, :]
                                    .bitcast(F32), 0.0)
                        r0 = h * VD + br * HD
                        nc.sync.dma_start(
                            out=kf[32 * s:32 * s + 32, :, :],
                            in_=kt_s[r0:r0 + HD, :]
                            .rearrange("p (k t) -> p k t", t=128).bitcast(F32R),
                        )
                        nc.sync.dma_start(
                            out=vh[:, :, 0:VD],
                            in_=v_s[:, h * VD:(h + 1) * VD]
                            .rearrange("(k p) v -> p k v", p=128).bitcast(F32R),
                        )
                        hp = h // 2
                        nc.sync.dma_start(
                            out=qp,
                            in_=qt_s[hp * 128:(hp + 1) * 128, :].bitcast(F32R),
                        )

                    def emit_S(pi):
                        cidx, qb, g, first, last = packs[pi]
                        if first and qb == 0:
                            emit_setup(cidx)
                        kf, qp = kfats[cidx % 2], qps[cidx % 2]
                        n = len(g)
                        sps = psS.tile([128, 1536], F32, tag="s")
                        for i, kt in enumerate(g):
                            nc.tensor.matmul(
                                sps[:, i * 512:(i + 1) * 512],
                                _r(kf[:, kt, :]),
                                _r(qp[:, qb * 512:(qb + 1) * 512]),
                                start=True, stop=True,
                            )
                        ex = expp.tile([128, 1536], F32R, tag="e")
                        nc.scalar.activation(
                            ex[:, 0:n * 512], sps[:, 0:n * 512],
                            mybir.ActivationFunctionType.Exp, scale=SCALE,
                        )
                        st["ex"][pi] = ex

                    def emit_PV(pi):
                        cidx, qb, g, first, last = packs[pi]

                        h, br, s = combo_hbr(cidx)
                        vh = vhs[cidx % 2]
                        ex = st["ex"].pop(pi)
                        if first:
                            st["o"][(cidx, qb)] = psO.tile([65, 512], F32, tag="o", name="o_ps")
                        o_ps = st["o"][(cidx, qb)]
                        for i, kt in enumerate(g):
                            nc.tensor.matmul(
                                o_ps,
                                _r(vh[:, kt, :]),
                                ex[:, i * 512:(i + 1) * 512],
                                start=(kt == 0),
                                stop=(kt == KT - 1),
                            )
                        if last and br == 0:
                            o_ps = st["o"].pop((cidx, qb))
                            sl = slice(qb * 512, (qb + 1) * 512)
                            stag_o = stagp.tile([VD, 512], BF16, tag="so")
                            nc.vector.tensor_copy(stag_o, o_ps[0:VD, :])
                            nc.sync.dma_start(
                                out=o1_s[h * VD:(h + 1) * VD, sl], in_=stag_o)
                            stag_z = stagp.tile([65, 512], F32, tag="sz")
                            nc.vector.tensor_copy(
                                stag_z[VD:65, :], o_ps[VD:65, :])
                            nc.sync.dma_start(
                                out=z1all[h:h + 1, sl], in_=stag_z[VD:65, :])
                        elif last:
                            # br1: hoisted combine, staggered so the PE-queue
                            # matmuls never starve the exp stream
                            o_ps = st["o"].pop((cidx, qb))
                            sl = slice(qb * 512, (qb + 1) * 512)
                            rsl = slice(h * VD, (h + 1) * VD)
                            nh = st["nh"] = st.get("nh", -1) + 1
                            trowf, u2f = trowfs[nh % 2], u2fs[nh % 2]
                            box = {}

                            def hoist_a(h=h, sl=sl, rsl=rsl, o_ps=o_ps,
                                        trowf=trowf, box=box):
                                zp0 = hscp.tile([1, 512], F32, tag="zp0",
                                                name="zp0")
                                nc.sync.dma_start(
                                    out=zp0, in_=z1all[h:h + 1, sl])
                                o1t = hscp.tile([VD, 512], BF16, tag="o1t",
                                                name="o1t")
                                nc.sync.dma_start(out=o1t, in_=o1_s[rsl, sl])
                                z2c = hscp.tile([65, 512], F32, tag="z2c",
                                                name="z2c")
                                nc.vector.tensor_copy(
                                    z2c[VD:65, :], o_ps[VD:65, :])
                                z2p0 = hscp.tile([1, 512], F32, tag="z2p0",
                                                 name="z2p0")
                                nc.sync.dma_start(
                                    out=z2p0, in_=z2c[VD:65, :])
                                rz2 = hscp.tile([1, 512], F32, tag="rz2",
                                                name="rz2")
                                rzs = hscp.tile([1, 512], F32, tag="rzs",
                                                name="rzs")
                                nc.vector.reciprocal_approx_accurate(
                                    out=rz2, in_=z2p0, scratch=rzs)
                                nc.vector.scalar_tensor_tensor(
                                    out=trowf[0:1, :], in0=zp0,
                                    scalar=float(lam), in1=rz2,
                                    op0=mybir.AluOpType.mult,
                                    op1=mybir.AluOpType.mult)
                                tbc = psO.tile([128, 512], F32, tag="o",
                                               name="tbc")
                                nc.tensor.matmul(
                                    tbc, padrow0, trowf,
                                    start=True, stop=True)
                                box["tbc"] = tbc
                                box["o1t"] = o1t
                                box["zp0"] = zp0

                            def hoist_b(h=h, sl=sl, rsl=rsl, o_ps=o_ps,
                                        u2f=u2f, box=box):
                                tbc, o1t, zp0 = (box["tbc"], box["o1t"],
                                                 box["zp0"])
                                o2s = hscp.tile([VD, 512], F32, tag="o2s",
                                                name="o2s")
                                nc.vector.tensor_copy(o2s, o_ps[0:VD, :])
                                o2x = hscp.tile([VD, 512], F32, tag="o2x",
                                                name="o2x")
                                nc.vector.tensor_mul(o2x, o2s, tbc[0:VD, :])
                                ut = hscp.tile([VD, 512], BF16, tag="ut",
                                               name="ut")
                                nc.vector.tensor_sub(ut, o1t, o2x)
                                nc.sync.dma_start(out=u_s[rsl, sl], in_=ut)
                                nc.vector.tensor_mul(u2f[0:VD, :], ut, ut)
                                mps = psO.tile([128, 512], F32, tag="o",
                                               name="mps")
                                nc.tensor.matmul(
                                    mps, padcol, u2f, start=True, stop=True)
                                ze = hscp.tile([1, 512], F32, tag="ze",
                                               name="ze")
                                nc.vector.tensor_scalar_mul(
                                    ze, zp0, float(EPS ** 0.5))
                                zsq = hscp.tile([1, 512], F32, tag="zsq",
                                                name="zsq")
                                nc.vector.tensor_mul(zsq, ze, ze)
                                arg0 = hscp.tile([1, 512], F32, tag="arg0",
                                                 name="arg0")
                                nc.vector.scalar_tensor_tensor(
                                    out=arg0, in0=mps[0:1, :],
                                    scalar=1.0 / VD, in1=zsq,
                                    op0=mybir.AluOpType.mult,
                                    op1=mybir.AluOpType.add)
                                nc.sync.dma_start(
                                    out=argall[h:h + 1, sl], in_=arg0)

                            sched.setdefault(pi + 2, []).append(hoist_a)
                            sched.setdefault(pi + 5, []).append(hoist_b)

                    emit_S(0)
                    for pi in range(len(packs) - 1):
                        emit_S(pi + 1)
                        emit_PV(pi)
                        for fn in sched.pop(pi, []):
                            fn()
                    emit_PV(len(packs) - 1)
                    for kk in sorted(sched):
                        for fn in sched[kk]:
                            fn()

                # ============ phase C: tail (norm + proj) ============
                with tc.tile_pool(name="row_p", bufs=1) as rowp:
                    rr0f = rowp.tile([128, NQ], F32R, tag="rr0f")
                    nc.vector.memset(rr0f.bitcast(F32), 0.0)
                    with tc.tile_pool(name="rowW", bufs=3) as roww:
                        sd = roww.tile([H, NQ], F32, tag="t")
                        nc.scalar.activation(
                            sd, argall, mybir.ActivationFunctionType.Sqrt)
                        rrt = roww.tile([H, NQ], F32, tag="t")
                        scr2 = roww.tile([H, NQ], F32, tag="t")
                        nc.vector.reciprocal_approx_accurate(
                            out=rrt, in_=sd, scratch=scr2)
                        nc.vector.tensor_copy(rr0f[0:H, :], rrt)

                    # o_n = u * rr * weff -> onstore [128, CIN, NQ] bf16
                    onstore = rowp.tile([128, CIN, NQ], BF16, tag="onstore")
                    with (
                        tc.tile_pool(name="inN", bufs=4) as inp,
                        tc.tile_pool(name="scrN", bufs=4) as scrp,
                        tc.tile_pool(name="psN", bufs=2, space="PSUM") as psN,
                    ):
                        for h in range(H):
                            for qb in range(4):
                                sl = slice(qb * 512, (qb + 1) * 512)
                                rsl = slice(h * VD, (h + 1) * VD)
                                ut = inp.tile([VD, 512], BF16, tag="ut")
                                nc.sync.dma_start(out=ut, in_=u_s[rsl, sl])
                                rrbc = psN.tile([128, 512], F32, tag="rrbc")
                                nc.tensor.matmul(
                                    rrbc, selfat[:, h, :], rr0f[:, sl],
                                    start=True, stop=True)
                                on = scrp.tile([VD, 512], F32, tag="on")
                                nc.vector.tensor_mul(on, ut, rrbc[0:VD, :])
                                if h % 2 == 0:
                                    nc.vector.tensor_scalar_mul(
                                        onstore[0:VD, h // 2, sl], on, weff_t)
                                else:
                                    onb = scrp.tile([VD, 512], BF16, tag="onb")
                                    nc.vector.tensor_scalar_mul(
                                        onb, on, weff_t)
                                    nc.sync.dma_start(
                                        out=onstore[VD:128, h // 2, sl],
                                        in_=onb)

                    # ============ phase D: proj (bf16, K=128) ============
                    with (
                        tc.tile_pool(name="wp_p", bufs=1) as wpp,
                        tc.tile_pool(name="yd_p", bufs=3) as ydp,
                        tc.tile_pool(name="psY", bufs=2, space="PSUM") as psY,
                    ):
                        wpb = wpp.tile([128, CIN, DIM], BF16)
                        nc.sync.dma_start(
                            out=wpb,
                            in_=wpbt[:, :].rearrange("v (c n) -> v c n", c=CIN))
                        bb = wpp.tile([128, DIM], F32)
                        nc.sync.dma_start(out=bb, in_=biasbc[:, :])
                        for qt in range(NQ // 128):
                            yps = psY.tile([128, 1024], F32, tag="y")
                            for sb in range(2):
                                for ci in range(CIN):
                                    nc.tensor.matmul(
                                        yps[:, sb * 512:(sb + 1) * 512],
                                        onstore[:, ci, qt * 128:(qt + 1) * 128],
                                        wpb[:, ci, sb * 512:(sb + 1) * 512],
                                        start=(ci == 0),
                                        stop=(ci == CIN - 1),
                                    )
                            yd = ydp.tile([128, 1024], F32, tag="yd")
                            nc.vector.tensor_add(yd, yps, bb)
                            nc.sync.dma_start(
                                out=y[qt * 128:(qt + 1) * 128, :], in_=yd)
    nc.finalize()
    return nc


def _make_inputs(x, w_qkv, w_proj, b_proj, sub_norm_w):
    wqkvt = np.ascontiguousarray(np.asarray(w_qkv, np.float32).T)
    wprojt = np.ascontiguousarray(np.asarray(w_proj, np.float32).T)  # [c, out]
    # proj weights: partition (h%2)*64+vd, col (h//2)*DIM+out
    wpbt = np.ascontiguousarray(
        wprojt.reshape(CIN, 2, VD, DIM).transpose(1, 2, 0, 3)
        .reshape(128, CIN * DIM)).astype(ml_dtypes.bfloat16)
    biasbc = np.ascontiguousarray(
        np.tile(np.asarray(b_proj, np.float32).reshape(1, DIM), (128, 1)))
    # selfat[:, h, :]: [128,128]; rows 0-15 hold one-hot h -> cols 0-63
    selp = np.zeros((128, H, 128), np.float32)
    for h in range(H):
        selp[h, h, 0:VD] = 1.0
    selp = np.ascontiguousarray(selp.reshape(128, H * 128))
    weff = (np.asarray(sub_norm_w, np.float32)
            * (1.0 - LAMBDA_INIT)).reshape(VD, 1)
    return wqkvt, wpbt, biasbc, weff, selp


def kernel(x, w_qkv, w_proj, b_proj, lambda_q1, lambda_k1, lambda_q2,
           lambda_k2, sub_norm_w):
    x = np.asarray(x, np.float32)
    lam = float(
        np.exp(np.sum(np.float64(lambda_q1) * np.float64(lambda_k1)))
        - np.exp(np.sum(np.float64(lambda_q2) * np.float64(lambda_k2)))
        + LAMBDA_INIT
    )
    wqkvt, wpbt, biasbc, weff, selp = _make_inputs(
        x, w_qkv, w_proj, b_proj, sub_norm_w)

    key = round(lam, 12)
    if key not in _CACHE:
        _CACHE[key] = build_nc(lam)
    nc = _CACHE[key]

    in_maps = []
    for c in range(NCORES):
        b, half = c // 2, c % 2
        xt = np.asarray(x[b].T)  # [DIM, N]
        if half == 1:  # query rows first
            xt = np.concatenate([xt[:, NQ:], xt[:, :NQ]], axis=1)
        in_maps.append({
            "xbt": np.ascontiguousarray(xt),
            "wqkvt": wqkvt,
            "wpbt": wpbt,
            "biasbc": biasbc,
            "weff": weff,
            "selp": selp,
        })
    res = run_bass_kernel_spmd(nc, in_maps, list(range(NCORES)))
    out = np.empty((B, N, DIM), np.float32)
    for c in range(NCORES):
        b, half = c // 2, c % 2
        out[b, half * NQ:(half + 1) * NQ, :] = res.results[c]["y"]
    return out


# revision 20
# speedup vs baseline: 2.2120x; 1.0200x over previous
"""DiffAttention TRN2 kernel: 8-way (batch x seq-half) sharded, zero collectives.

v2 pipeline (ACT-exp is the roofline: 268M exps/core ~= 2.05ms):
  - Phase A: qkv projections to DRAM scratch (Q^T, K^T, V), token-quartered.
  - Phase B attention, combo-major ((head,branch) = combo, in 4 strip classes
    so zero-padded K tiles never need re-zeroing inside a class). S matmuls
    use kfat [128,128] lhsT = K^T rows at partition strip 32s, zeros
    elsewhere -> every phase A+B matmul runs in the same 128x128 PE tiling
    mode (no PE drain/reconfig). 3-kt S packs -> one [128,1536] PSUM tile ->
    single exp ACT (1.49us, ACT ~100% busy). Emission order S(g+1) before
    PV(g) so the in-order PE queue never stalls on the exp semaphore.
    PV lhsT = V_aug [128,65] (ones col -> softmax denominators free),
    accumulated over 32 key tiles into o_ps [65,512]; drained to bf16
    o1store/o2store (partitions 0-63) + Z rows DMA'd to base-0 z tiles.
  - Phase C tail: wide [16,2048] row math (reciprocal_approx_accurate,
    single Sqrt table switch), sel-matrix ones-matmul row broadcasts,
    mode-batched sub-loops.
  - Phase D proj: bf16 weights (host-cast), K=64 per-head contraction,
    bias added via host-tiled broadcast tensor.
"""

import sys

import numpy as np

for p in ("/opt/trn_rl_repo",):
    if p not in sys.path:
        sys.path.insert(0, p)

import ml_dtypes

import concourse.bass as bass
import concourse.bacc as bacc_mod
import concourse.mybir as mybir
from concourse.bass_utils import run_bass_kernel_spmd
from concourse.tile import TileContext

F32 = mybir.dt.float32
F32R = mybir.dt.float32r
BF16 = mybir.dt.bfloat16

B, N, DIM, H, HD = 4, 4096, 1024, 16, 32
VD = 2 * HD  # 64, per-head v dim
NQ = 2048  # query rows per core
KT = N // 128  # 32 key tiles
CIN = DIM // 128  # 8 contraction tiles
NCORES = 8
LAMBDA_INIT = 0.2
EPS = 1e-5
SCALE = HD ** -0.5

_CACHE = {}


def _r(ap):
    return ap.bitcast(F32R)


def build_nc(lam: float):
    nc = bacc_mod.Bacc(None, target_bir_lowering=False)

    xbt = nc.declare_dram_parameter("xbt", [DIM, N], F32, isOutput=False)
    wqkvt = nc.declare_dram_parameter("wqkvt", [DIM, 3 * DIM], F32, isOutput=False)
    wpbt = nc.declare_dram_parameter("wpbt", [128, CIN * DIM], BF16, isOutput=False)
    biasbc = nc.declare_dram_parameter("biasbc", [128, DIM], F32, isOutput=False)
    weff = nc.declare_dram_parameter("weff", [VD, 1], F32, isOutput=False)
    selp = nc.declare_dram_parameter("selp", [128, H * 128], F32, isOutput=False)
    y = nc.declare_dram_parameter("y", [NQ, DIM], F32, isOutput=True)

    qt_s = nc.dram_tensor("qt_scratch", [DIM, NQ], F32)
    o1_s = nc.dram_tensor("o1_scratch", [H * VD, NQ], BF16)
    u_s = nc.dram_tensor("u_scratch", [H * VD, NQ], BF16)
    kt_s = nc.dram_tensor("kt_scratch", [DIM, N], F32)
    v_s = nc.dram_tensor("v_scratch", [N, DIM], F32)

    with nc.allow_low_precision(reason="f32r bit-identical fp32; bf16 stores"), \
         TileContext(nc) as tc:
        with (
            tc.tile_pool(name="const", bufs=1) as constp,
            tc.tile_pool(name="store", bufs=1) as storep,
        ):
            # padrow0: row 0 = [1]*VD pad 0 -> broadcast partition-0 row
            padrow0 = constp.tile([128, 128], F32R)
            nc.vector.memset(padrow0.bitcast(F32), 0.0)
            nc.vector.memset(padrow0[0:1, 0:VD].bitcast(F32), 1.0)
            # padcol: col 0 = ones on partitions 0-63 -> colsum over vd
            padcol = constp.tile([128, 128], F32R)
            nc.vector.memset(padcol.bitcast(F32), 0.0)
            nc.vector.memset(padcol[0:VD, 0:1].bitcast(F32), 1.0)
            # selfat[:, h, :]: [128, 128] one-hot padded broadcast matrices
            selfat = constp.tile([128, H, 128], F32R)
            nc.sync.dma_start(
                out=selfat,
                in_=selp[:, :].rearrange("p (h v) -> p h v", v=128)
                .bitcast(F32R))
            weff_t = constp.tile([VD, 1], F32)
            nc.sync.dma_start(out=weff_t, in_=weff[:, :])

            # persistent row stores (base 0)
            z1all = storep.tile([H, NQ], F32)
            argall = storep.tile([H, NQ], F32)

            if True:
                # ================= phase A: qkv =================
                with (
                    tc.tile_pool(name="xbt_p", bufs=2) as xbtp,
                    tc.tile_pool(name="wq_p", bufs=4) as wqp,
                    tc.tile_pool(name="wv_p", bufs=2) as wvp,
                    tc.tile_pool(name="drain_p", bufs=3) as drp,
                    tc.tile_pool(name="psA", bufs=3, space="PSUM") as psA,
                    tc.tile_pool(name="psAv", bufs=2, space="PSUM") as psAv,
                ):
                    for tq in range(4):  # token quarters of 1024
                        xb = xbtp.tile([128, CIN, 1024], F32R, tag="xb")
                        nc.sync.dma_start(
                            out=xb,
                            in_=xbt[:, tq * 1024:(tq + 1) * 1024]
                            .rearrange("(t p) n -> p t n", p=128).bitcast(F32R),
                        )
                        for co in range(2 * CIN):  # 0..7 Q, 8..15 K
                            is_q = co < CIN
                            if is_q and tq >= 2:
                                continue
                            ps = psA.tile([128, 1024], F32, tag="ps")
                            for ci in range(CIN):
                                wt = wqp.tile([128, 128], F32R, tag="w")
                                nc.sync.dma_start(
                                    out=wt,
                                    in_=wqkvt[ci * 128:(ci + 1) * 128,
                                              co * 128:(co + 1) * 128]
                                    .bitcast(F32R),
                                )
                                for sb in range(2):
                                    nc.tensor.matmul(
                                        ps[:, sb * 512:(sb + 1) * 512],
                                        _r(wt),
                                        _r(xb[:, ci, sb * 512:(sb + 1) * 512]),
                                        start=(ci == 0),
                                        stop=(ci == CIN - 1),
                                    )
                            dr = drp.tile([128, 1024], F32, tag="dr")
                            nc.vector.tensor_copy(dr, ps)
                            dst = qt_s if is_q else kt_s
                            coo = co if is_q else co - CIN
                            nc.sync.dma_start(
                                out=dst[coo * 128:(coo + 1) * 128,
                                        tq * 1024:(tq + 1) * 1024],
                                in_=dr,
                            )
                        for cc in range(DIM // 512):
                            wv = wvp.tile([128, CIN, 512], F32R, tag="wv")
                            nc.sync.dma_start(
                                out=wv,
                                in_=wqkvt[:, 2 * DIM + cc * 512:
                                          2 * DIM + (cc + 1) * 512]
                                .rearrange("(t p) n -> p t n", p=128)
                                .bitcast(F32R),
                            )
                            for kt in range(8):
                                psv = psAv.tile([128, 512], F32, tag="psv")
                                for ci in range(CIN):
                                    nc.tensor.matmul(
                                        psv,
                                        _r(xb[:, ci, kt * 128:(kt + 1) * 128]),
                                        _r(wv[:, ci, :]),
                                        start=(ci == 0),
                                        stop=(ci == CIN - 1),
                                    )
                                drv = drp.tile([128, 512], F32, tag="drv")
                                if kt % 2 == 0:
                                    nc.vector.tensor_copy(drv, psv)
                                else:
                                    nc.scalar.activation(
                                        drv, psv,
                                        mybir.ActivationFunctionType.Copy)
                                nc.sync.dma_start(
                                    out=v_s[tq * 1024 + kt * 128:
                                            tq * 1024 + (kt + 1) * 128,
                                            cc * 512:(cc + 1) * 512],
                                    in_=drv,
                                )

                # ================= phase B: attention =================
                with (
                    tc.tile_pool(name="kfat_p", bufs=1) as kfatp,
                    tc.tile_pool(name="vh_p", bufs=1) as vhp,
                    tc.tile_pool(name="qp_p", bufs=1) as qpp,
                    tc.tile_pool(name="exp_p", bufs=3) as expp,
                    tc.tile_pool(name="stag_p", bufs=2) as stagp,
                    tc.tile_pool(name="hsc_p", bufs=2) as hscp,
                    tc.tile_pool(name="psS", bufs=2, space="PSUM") as psS,
                    tc.tile_pool(name="psO", bufs=2, space="PSUM") as psO,
                ):
                    kfats = [kfatp.tile([128, KT, 128], F32R, tag=f"k{i}", name=f"kfat{i}")
                             for i in range(2)]
                    vhs = [vhp.tile([128, KT, 65], F32R, tag=f"v{i}", name=f"vh{i}")
                           for i in range(2)]
                    qps = [qpp.tile([128, NQ], F32R, tag=f"q{i}", name=f"qp{i}")
                           for i in range(2)]
                    trowfs = [hscp.tile([128, 512], F32R, tag=f"tr{i}",
                                        name=f"trowf{i}") for i in range(2)]
                    u2fs = [hscp.tile([128, 512], F32R, tag=f"u2{i}",
                                      name=f"u2f{i}") for i in range(2)]
                    for i in range(2):
                        nc.vector.memset(trowfs[i].bitcast(F32), 0.0)
                        nc.vector.memset(u2fs[i][VD:128, :].bitcast(F32), 0.0)
                    for i in range(2):
                        nc.vector.memset(kfats[i].bitcast(F32), 0.0)
                        nc.vector.memset(vhs[i][:, :, VD:65].bitcast(F32), 1.0)

                    groups = [list(range(3 * g, 3 * g + 3)) for g in range(10)]
                    groups.append([30, 31])
                    packs = []
                    for cidx in range(32):
                        for qb in range(4):
                            for gi, g in enumerate(groups):
                                packs.append(
                                    (cidx, qb, g, gi == 0, gi == len(groups) - 1))

                    st = {"o": {}, "ex": {}}
                    sched = {}

                    def combo_hbr(idx):
                        s, pc = idx // 8, idx % 8
                        return 2 * pc + (s >> 1), s & 1, s

                    def emit_setup(idx):
                        h, br, s = combo_hbr(idx)
                        kf, vh, qp = kfats[idx % 2], vhs[idx % 2], qps[idx % 2]
                        if idx in (8, 16, 24) and idx % 2 == 0:
                            pass
                        if idx in (8, 16, 24):
                            s_old = (idx - 1) // 8
                            for i in range(2):
                                nc.vector.memset(
                                    kfats[i][32 * s_old:32 * s_old + 32, :, :]
                                    .bitcast(F32), 0.0)
                        r0 = h * VD + br * HD
                        nc.sync.dma_start(
                            out=kf[32 * s:32 * s + 32, :, :],
                            in_=kt_s[r0:r0 + HD, :]
                            .rearrange("p (k t) -> p k t", t=128).bitcast(F32R),
                        )
                        nc.sync.dma_start(
                            out=vh[:, :, 0:VD],
                            in_=v_s[:, h * VD:(h + 1) * VD]
                            .rearrange("(k p) v -> p k v", p=128).bitcast(F32R),
                        )
                        hp = h // 2
                        nc.sync.dma_start(
                            out=qp,
                            in_=qt_s[hp * 128:(hp + 1) * 128, :].bitcast(F32R),
                        )

                    def emit_S(pi):
                        cidx, qb, g, first, last = packs[pi]
                        if first and qb == 0:
                            emit_setup(cidx)
                        kf, qp = kfats[cidx % 2], qps[cidx % 2]
                        n = len(g)
                        sps = psS.tile([128, 1536], F32, tag="s")
                        for i, kt in enumerate(g):
                            nc.tensor.matmul(
                                sps[:, i * 512:(i + 1) * 512],
                                _r(kf[:, kt, :]),
                                _r(qp[:, qb * 512:(qb + 1) * 512]),
                                start=True, stop=True,
                            )
                        ex = expp.tile([128, 1536], F32R, tag="e")
                        nc.scalar.activation(
                            ex[:, 0:n * 512], sps[:, 0:n * 512],
                            mybir.ActivationFunctionType.Exp, scale=SCALE,
                        )
                        st["ex"][pi] = ex

                    def emit_PV(pi):
                        cidx, qb, g, first, last = packs[pi]

                        h, br, s = combo_hbr(cidx)
                        vh = vhs[cidx % 2]
                        ex = st["ex"].pop(pi)
                        if first:
                            st["o"][(cidx, qb)] = psO.tile([65, 512], F32, tag="o", name="o_ps")
                        o_ps = st["o"][(cidx, qb)]
                        for i, kt in enumerate(g):
                            nc.tensor.matmul(
                                o_ps,
                                _r(vh[:, kt, :]),
                                ex[:, i * 512:(i + 1) * 512],
                                start=(kt == 0),
                                stop=(kt == KT - 1),
                            )
                        if last and br == 0:
                            o_ps = st["o"].pop((cidx, qb))
                            sl = slice(qb * 512, (qb + 1) * 512)
                            stag_o = stagp.tile([VD, 512], BF16, tag="so")
                            nc.vector.tensor_copy(stag_o, o_ps[0:VD, :])
                            nc.sync.dma_start(
                                out=o1_s[h * VD:(h + 1) * VD, sl], in_=stag_o)
                            stag_z = stagp.tile([65, 512], F32, tag="sz")
                            nc.vector.tensor_copy(
                                stag_z[VD:65, :], o_ps[VD:65, :])
                            nc.sync.dma_start(
                                out=z1all[h:h + 1, sl], in_=stag_z[VD:65, :])
                        elif last:
                            # br1: hoisted combine, staggered so the PE-queue
                            # matmuls never starve the exp stream
                            o_ps = st["o"].pop((cidx, qb))
                            sl = slice(qb * 512, (qb + 1) * 512)
                            rsl = slice(h * VD, (h + 1) * VD)
                            nh = st["nh"] = st.get("nh", -1) + 1
                            trowf, u2f = trowfs[nh % 2], u2fs[nh % 2]
                            box = {}

                            def hoist_a(h=h, sl=sl, rsl=rsl, o_ps=o_ps,
                                        trowf=trowf, box=box):
                                zp0 = hscp.tile([1, 512], F32, tag="zp0",
                                                name="zp0")
                                nc.sync.dma_start(
                                    out=zp0, in_=z1all[h:h + 1, sl])
                                o1t = hscp.tile([VD, 512], BF16, tag="o1t",
                                                name="o1t")
                                nc.sync.dma_start(out=o1t, in_=o1_s[rsl, sl])
                                z2c = hscp.tile([65, 512], F32, tag="z2c",
                                                name="z2c")
                                nc.vector.tensor_copy(
                                    z2c[VD:65, :], o_ps[VD:65, :])
                                z2p0 = hscp.tile([1, 512], F32, tag="z2p0",
                                                 name="z2p0")
                                nc.sync.dma_start(
                                    out=z2p0, in_=z2c[VD:65, :])
                                rz2 = hscp.tile([1, 512], F32, tag="rz2",
                                                name="rz2")
                                rzs = hscp.tile([1, 512], F32, tag="rzs",
                                                name="rzs")
                                nc.vector.reciprocal_approx_accurate(
                                    out=rz2, in_=z2p0, scratch=rzs)
                                nc.vector.scalar_tensor_tensor(
                                    out=trowf[0:1, :], in0=zp0,
                                    scalar=float(lam), in1=rz2,
                                    op0=mybir.AluOpType.mult,
                                    op1=mybir.AluOpType.mult)
                                tbc = psO.tile([128, 512], F32, tag="o",
                                               name="tbc")
                                nc.tensor.matmul(
                                    tbc, padrow0, trowf,
                                    start=True, stop=True)
                                box["tbc"] = tbc
                                box["o1t"] = o1t
                                box["zp0"] = zp0

                            def hoist_b(h=h, sl=sl, rsl=rsl, o_ps=o_ps,
                                        u2f=u2f, box=box):
                                tbc, o1t, zp0 = (box["tbc"], box["o1t"],
                                                 box["zp0"])
                                o2s = hscp.tile([VD, 512], F32, tag="o2s",
                                                name="o2s")
                                nc.vector.tensor_copy(o2s, o_ps[0:VD, :])
                                o2x = hscp.tile([VD, 512], F32, tag="o2x",
                                                name="o2x")
                                nc.vector.tensor_mul(o2x, o2s, tbc[0:VD, :])
                                ut = hscp.tile([VD, 512], BF16, tag="ut",
                                               name="ut")
                                nc.vector.tensor_sub(ut, o1t, o2x)
                                nc.sync.dma_start(out=u_s[rsl, sl], in_=ut)
                                nc.vector.tensor_mul(u2f[0:VD, :], ut, ut)
                                mps = psO.tile([128, 512], F32, tag="o",
                                               name="mps")
                                nc.tensor.matmul(
                                    mps, padcol, u2f, start=True, stop=True)
                                ze = hscp.tile([1, 512], F32, tag="ze",
                                               name="ze")
                                nc.vector.tensor_scalar_mul(
                                    ze, zp0, float(EPS ** 0.5))
                                zsq = hscp.tile([1, 512], F32, tag="zsq",
                                                name="zsq")
                                nc.vector.tensor_mul(zsq, ze, ze)
                                arg0 = hscp.tile([1, 512], F32, tag="arg0",
                                                 name="arg0")
                                nc.vector.scalar_tensor_tensor(
                                    out=arg0, in0=mps[0:1, :],
                                    scalar=1.0 / VD, in1=zsq,
                                    op0=mybir.AluOpType.mult,
                                    op1=mybir.AluOpType.add)
                                nc.sync.dma_start(
                                    out=argall[h:h + 1, sl], in_=arg0)

                            sched.setdefault(pi + 2, []).append(hoist_a)
                            sched.setdefault(pi + 5, []).append(hoist_b)

                    emit_S(0)
                    for pi in range(len(packs) - 1):
                        emit_S(pi + 1)
                        emit_PV(pi)
                        for fn in sched.pop(pi, []):
                            fn()
                    emit_PV(len(packs) - 1)
                    for kk in sorted(sched):
                        for fn in sched[kk]:
                            fn()

                # ============ phase C: tail (norm + proj) ============
                with tc.tile_pool(name="row_p", bufs=1) as rowp:
                    rr0f = rowp.tile([128, NQ], F32R, tag="rr0f")
                    nc.vector.memset(rr0f.bitcast(F32), 0.0)
                    with tc.tile_pool(name="rowW", bufs=3) as roww:
                        sd = roww.tile([H, NQ], F32, tag="t")
                        nc.scalar.activation(
                            sd, argall, mybir.ActivationFunctionType.Sqrt)
                        rrt = roww.tile([H, NQ], F32, tag="t")
                        scr2 = roww.tile([H, NQ], F32, tag="t")
                        nc.vector.reciprocal_approx_accurate(
                            out=rrt, in_=sd, scratch=scr2)
                        nc.vector.tensor_copy(rr0f[0:H, :], rrt)

                    # o_n = u * rr * weff -> onstore [128, CIN, NQ] bf16
                    onstore = rowp.tile([128, CIN, NQ], BF16, tag="onstore")
                    with (
                        tc.tile_pool(name="inN", bufs=4) as inp,
                        tc.tile_pool(name="scrN", bufs=4) as scrp,
                        tc.tile_pool(name="psN", bufs=2, space="PSUM") as psN,
                    ):
                        for h in range(H):
                            for qb in range(4):
                                sl = slice(qb * 512, (qb + 1) * 512)
                                rsl = slice(h * VD, (h + 1) * VD)
                                ut = inp.tile([VD, 512], BF16, tag="ut")
                                nc.sync.dma_start(out=ut, in_=u_s[rsl, sl])
                                rrbc = psN.tile([128, 512], F32, tag="rrbc")
                                nc.tensor.matmul(
                                    rrbc, selfat[:, h, :], rr0f[:, sl],
                                    start=True, stop=True)
                                on = scrp.tile([VD, 512], F32, tag="on")
                                nc.vector.tensor_mul(on, ut, rrbc[0:VD, :])
                                if h % 2 == 0:
                                    nc.vector.tensor_scalar_mul(
                                        onstore[0:VD, h // 2, sl], on, weff_t)
                                else:
                                    onb = scrp.tile([VD, 512], BF16, tag="onb")
                                    nc.vector.tensor_scalar_mul(
                                        onb, on, weff_t)
                                    nc.sync.dma_start(
                                        out=onstore[VD:128, h // 2, sl],
                                        in_=onb)

                    # ============ phase D: proj (bf16, K=128) ============
                    with (
                        tc.tile_pool(name="wp_p", bufs=1) as wpp,
                        tc.tile_pool(name="yd_p", bufs=3) as ydp,
                        tc.tile_pool(name="psY", bufs=2, space="PSUM") as psY,
                    ):
                        wpb = wpp.tile([128, CIN, DIM], BF16)
                        nc.sync.dma_start(
                            out=wpb,
                            in_=wpbt[:, :].rearrange("v (c n) -> v c n", c=CIN))
                        bb = wpp.tile([128, DIM], F32)
                        nc.sync.dma_start(out=bb, in_=biasbc[:, :])
                        for qt in range(NQ // 128):
                            yps = psY.tile([128, 1024], F32, tag="y")
                            for sb in range(2):
                                for ci in range(CIN):
                                    nc.tensor.matmul(
                                        yps[:, sb * 512:(sb + 1) * 512],
                                        onstore[:, ci, qt * 128:(qt + 1) * 128],
                                        wpb[:, ci, sb * 512:(sb + 1) * 512],
                                        start=(ci == 0),
                                        stop=(ci == CIN - 1),
                                    )
                            yd = ydp.tile([128, 1024], F32, tag="yd")
                            nc.vector.tensor_add(yd, yps, bb)
                            nc.sync.dma_start(
                                out=y[qt * 128:(qt + 1) * 128, :], in_=yd)
    nc.finalize()
    return nc


def _make_inputs(x, w_qkv, w_proj, b_proj, sub_norm_w):
    wqkvt = np.ascontiguousarray(np.asarray(w_qkv, np.float32).T)
    wprojt = np.ascontiguousarray(np.asarray(w_proj, np.float32).T)  # [c, out]
    # proj weights: partition (h%2)*64+vd, col (h//2)*DIM+out
    wpbt = np.ascontiguousarray(
        wprojt.reshape(CIN, 2, VD, DIM).transpose(1, 2, 0, 3)
        .reshape(128, CIN * DIM)).astype(ml_dtypes.bfloat16)
    biasbc = np.ascontiguousarray(
        np.tile(np.asarray(b_proj, np.float32).reshape(1, DIM), (128, 1)))
    # selfat[:, h, :]: [128,128]; rows 0-15 hold one-hot h -> cols 0-63
    selp = np.zeros((128, H, 128), np.float32)
    for h in range(H):
        selp[h, h, 0:VD] = 1.0
    selp = np.ascontiguousarray(selp.reshape(128, H * 128))
    weff = (np.asarray(sub_norm_w, np.float32)
            * (1.0 - LAMBDA_INIT)).reshape(VD, 1)
    return wqkvt, wpbt, biasbc, weff, selp


def kernel(x, w_qkv, w_proj, b_proj, lambda_q1, lambda_k1, lambda_q2,
           lambda_k2, sub_norm_w):
    x = np.asarray(x, np.float32)
    lam = float(
        np.exp(np.sum(np.float64(lambda_q1) * np.float64(lambda_k1)))
        - np.exp(np.sum(np.float64(lambda_q2) * np.float64(lambda_k2)))
        + LAMBDA_INIT
    )
    wqkvt, wpbt, biasbc, weff, selp = _make_inputs(
        x, w_qkv, w_proj, b_proj, sub_norm_w)

    key = round(lam, 12)
    if key not in _CACHE:
        _CACHE[key] = build_nc(lam)
    nc = _CACHE[key]

    in_maps = []
    for c in range(NCORES):
        b, half = c // 2, c % 2
        xt = np.asarray(x[b].T)  # [DIM, N]
        if half == 1:  # query rows first
            xt = np.concatenate([xt[:, NQ:], xt[:, :NQ]], axis=1)
        in_maps.append({
            "xbt": np.ascontiguousarray(xt),
            "wqkvt": wqkvt,
            "wpbt": wpbt,
            "biasbc": biasbc,
            "weff": weff,
            "selp": selp,
        })
    res = run_bass_kernel_spmd(nc, in_maps, list(range(NCORES)))
    out = np.empty((B, N, DIM), np.float32)
    for c in range(NCORES):
        b, half = c // 2, c % 2
        out[b, half * NQ:(half + 1) * NQ, :] = res.results[c]["y"]
    return out
